# revision 5
# baseline (speedup 1.0000x reference)
"""AGRU cell (antisymmetric GRU) forward on 8 TRN2 NeuronCores.

Data-parallel: batch 16384 is sharded 2048 rows/core; the six 1024x1024
weight matrices are replicated. No cross-core communication.

Everything on-device is computed in "hidden-major" (transposed) layout:
    zT = sigmoid(Wz @ xT + Uz @ hT + bz)        [H, B]
    rT = sigmoid(Wr @ xT + Ur @ hT + br)
    rhT = rT * hT
    dhT = tanh(Vh @ xT + A @ rhT + bh)
    outT = hT + eps * zT * dhT
so every matmul has the (pre-transposed, host-prepared) weight tile as the
stationary operand and xT/hT/rhT as the moving operand, and nothing ever
needs an on-device transpose.  The host transposes each core's [1024, 2048]
result back when assembling the full output.

Matmuls run in bf16 (1 cycle/row on TRN2 vs 4 for fp32) with fp32 PSUM
accumulation; the final residual add is done in fp32.
"""

import sys

sys.path.insert(0, "/opt/trn_rl_repo")

import numpy as np
import ml_dtypes

from contextlib import ExitStack

import concourse.bass as bass
import concourse.mybir as mybir
from concourse import bacc, tile
from concourse.bass import ds, ts
from concourse.bass_utils import run_bass_kernel_spmd

BF16 = mybir.dt.bfloat16
F32 = mybir.dt.float32
AFT = mybir.ActivationFunctionType
ALU = mybir.AluOpType

N_CORES = 8
BATCH = 16384
B = BATCH // N_CORES  # per-core batch shard (2048)
H = 1024  # hidden == input size
KC = H // 128  # contraction chunks (8)
JT = H // 128  # output row tiles (8)
NB = 4  # moving-dim (batch) blocks per psum bank
NBS = B // NB  # 512 columns per matmul
GAMMA = 0.01

_nc_cache = {}


def _build(eps: float):
    """Build + compile the single-core Tile program (same graph on all cores)."""
    nc = bacc.Bacc("TRN2", target_bir_lowering=False, debug=False)

    xT_d = nc.dram_tensor("xT", [128, KC, B], BF16, kind="ExternalInput")
    hT_d = nc.dram_tensor("hT", [128, KC, B], BF16, kind="ExternalInput")
    w_d = {
        name: nc.dram_tensor(name, [JT, 128, KC, 128], BF16, kind="ExternalInput")
        for name in ["wzT", "uzT", "wrT", "urT", "vhT", "aT"]
    }
    bias_d = nc.dram_tensor("biases", [128, 24], F32, kind="ExternalInput")
    out_d = nc.dram_tensor("out", [H, B], F32, kind="ExternalOutput")

    with tile.TileContext(nc) as tc, ExitStack() as ctx:
        singles = ctx.enter_context(tc.tile_pool(name="singles", bufs=1))
        wpool = ctx.enter_context(tc.tile_pool(name="wpool", bufs=10))
        psum = ctx.enter_context(tc.tile_pool(name="psum", bufs=8, space="PSUM"))
        actp = ctx.enter_context(tc.tile_pool(name="actp", bufs=6))
        tmpp = ctx.enter_context(tc.tile_pool(name="tmpp", bufs=4))
        outp = ctx.enter_context(tc.tile_pool(name="outp", bufs=3))

        xT = singles.tile([128, KC, B], BF16)
        hTb = singles.tile([128, KC, B], BF16)
        rhT = singles.tile([128, KC, B], BF16)
        bias_sb = singles.tile([128, 24], F32)

        for c in range(KC):
            nc.sync.dma_start(out=xT[:, c, :], in_=xT_d[:, c, :])
            nc.sync.dma_start(out=hTb[:, c, :], in_=hT_d[:, c, :])
        nc.sync.dma_start(out=bias_sb[:], in_=bias_d[:])

        def load_w(name, jt):
            w = wpool.tile([128, KC, 128], BF16, tag="w")
            nc.sync.dma_start(out=w[:], in_=w_d[name][jt])
            return w

        def gemm_pair(psums, wA, rhsA, wB, rhsB):
            # psums[nb] += wA[:,k,:].T @ rhsA[:,k,nb] summed over k, then wB/rhsB
            for k in range(KC):
                for nb in range(NB):
                    nc.tensor.matmul(
                        psums[nb][:],
                        wA[:, k, :],
                        rhsA[:, k, ds(nb * NBS, NBS)],
                        start=(k == 0),
                        stop=False,
                    )
            for k in range(KC):
                for nb in range(NB):
                    nc.tensor.matmul(
                        psums[nb][:],
                        wB[:, k, :],
                        rhsB[:, k, ds(nb * NBS, NBS)],
                        start=False,
                        stop=(k == KC - 1),
                    )

        # ---- phase 1: r gate (hidden-major), rhT = sigmoid(...) * hT ----
        for jt in range(JT):
            wr = load_w("wrT", jt)
            ur = load_w("urT", jt)
            ps = [
                psum.tile([128, NBS], F32, tag="ps", name=f"ps_r{jt}_{i}")
                for i in range(NB)
            ]
            gemm_pair(ps, wr, xT, ur, hTb)
            for nb in range(NB):
                rt = actp.tile([128, NBS], BF16, tag="act")
                nc.scalar.activation(
                    rt[:], ps[nb][:], AFT.Sigmoid, bias=bias_sb[:, 8 + jt : 9 + jt]
                )
                nc.vector.tensor_mul(
                    rhT[:, jt, ds(nb * NBS, NBS)],
                    rt[:],
                    hTb[:, jt, ds(nb * NBS, NBS)],
                )

        # ---- phase 2: z gate + delta_h + residual, one jt row-block at a time ----
        for jt in range(JT):
            wz = load_w("wzT", jt)
            uz = load_w("uzT", jt)
            vh = load_w("vhT", jt)
            at = load_w("aT", jt)
            psz = [
                psum.tile([128, NBS], F32, tag="ps", name=f"ps_z{jt}_{i}")
                for i in range(NB)
            ]
            gemm_pair(psz, wz, xT, uz, hTb)
            psd = [
                psum.tile([128, NBS], F32, tag="ps", name=f"ps_d{jt}_{i}")
                for i in range(NB)
            ]
            gemm_pair(psd, vh, xT, at, rhT)
            ot = outp.tile([128, B], F32, tag="out")
            for nb in range(NB):
                zt = actp.tile([128, NBS], BF16, tag="act")
                nc.scalar.activation(
                    zt[:], psz[nb][:], AFT.Sigmoid, bias=bias_sb[:, jt : jt + 1]
                )
                dt_ = actp.tile([128, NBS], BF16, tag="act")
                nc.scalar.activation(
                    dt_[:], psd[nb][:], AFT.Tanh, bias=bias_sb[:, 16 + jt : 17 + jt]
                )
                zdh = tmpp.tile([128, NBS], F32, tag="zdh")
                nc.vector.tensor_mul(zdh[:], zt[:], dt_[:])
                # out = (z*dh) * eps + h
                nc.vector.scalar_tensor_tensor(
                    ot[:, ds(nb * NBS, NBS)],
                    zdh[:],
                    float(eps),
                    hTb[:, jt, ds(nb * NBS, NBS)],
                    op0=ALU.mult,
                    op1=ALU.add,
                )
            nc.sync.dma_start(out=out_d[ts(jt, 128), :], in_=ot[:])

    nc.compile()
    return nc


def _get_nc(eps: float):
    key = float(eps)
    if key not in _nc_cache:
        _nc_cache[key] = _build(key)
    return _nc_cache[key]


def _block_weight(wT):
    # [1024, 1024] (contraction-major) -> [jt, p, c, j] st. blk[jt,p,c,j] = wT[c*128+p, jt*128+j]
    return np.ascontiguousarray(
        wT.reshape(KC, 128, JT, 128).transpose(2, 1, 0, 3)
    ).astype(ml_dtypes.bfloat16)


def _block_data(m):
    # per-core [B, 1024] -> [p, c, b] st. blk[p,c,b] = m[b, c*128+p]
    return np.ascontiguousarray(m.T.reshape(KC, 128, B).transpose(1, 0, 2))


def _prep_in_maps(x, h_prev, W_z, b_z, U_z, W_r, b_r, U_r, V_h, b_h, W_h):
    x16 = np.asarray(x, np.float32).astype(ml_dtypes.bfloat16)
    h16 = np.asarray(h_prev, np.float32).astype(ml_dtypes.bfloat16)

    A = W_h - W_h.T - GAMMA * np.eye(H, dtype=np.float32)
    shared = {
        "wzT": _block_weight(W_z.T),
        "uzT": _block_weight(U_z.T),
        "wrT": _block_weight(W_r.T),
        "urT": _block_weight(U_r.T),
        "vhT": _block_weight(V_h.T),
        "aT": _block_weight(A.T),
        "biases": np.ascontiguousarray(
            np.concatenate(
                [
                    b_z.reshape(JT, 128).T,
                    b_r.reshape(JT, 128).T,
                    b_h.reshape(JT, 128).T,
                ],
                axis=1,
            ).astype(np.float32)
        ),
    }
    in_maps = []
    for c in range(N_CORES):
        sl = slice(c * B, (c + 1) * B)
        in_maps.append(
            {"xT": _block_data(x16[sl]), "hT": _block_data(h16[sl]), **shared}
        )
    return in_maps


def run(inputs, trace=False):
    """Returns (full_output [16384,1024] f32, BassKernelResults)."""
    np_in = {k: np.asarray(v, np.float32) for k, v in inputs.items()}
    eps = float(np_in.pop("epsilon"))
    in_maps = _prep_in_maps(**np_in)
    nc = _get_nc(eps)
    res = run_bass_kernel_spmd(
        nc, in_maps, core_ids=list(range(N_CORES)), trace=trace
    )
    out = np.empty((BATCH, H), np.float32)
    for c in range(N_CORES):
        out[c * B : (c + 1) * B, :] = res.results[c]["out"].T
    return out, res


def kernel(**inputs) -> np.ndarray:
    out, _ = run(inputs, trace=False)
    return out


# revision 10
# speedup vs baseline: 1.0108x; 1.0108x over previous
"""AGRU cell (antisymmetric GRU) forward on 8 TRN2 NeuronCores.

Data-parallel: batch 16384 is sharded 2048 rows/core; the six 1024x1024
weight matrices are replicated. No cross-core communication.

Everything on-device is computed in "hidden-major" (transposed) layout:
    zT = sigmoid(Wz @ xT + Uz @ hT + bz)        [H, B]
    rT = sigmoid(Wr @ xT + Ur @ hT + br)
    rhT = rT * hT
    dhT = tanh(Vh @ xT + A @ rhT + bh)
    outT = hT + eps * zT * dhT
so every matmul has the (pre-transposed, host-prepared) weight tile as the
stationary operand and xT/hT/rhT as the moving operand, and nothing ever
needs an on-device transpose.  The host transposes each core's [1024, 2048]
result back when assembling the full output.

Matmuls run in bf16 (1 cycle/row on TRN2 vs 4 for fp32) with fp32 PSUM
accumulation; the final residual add is done in fp32.
"""

import sys

sys.path.insert(0, "/opt/trn_rl_repo")

import numpy as np
import ml_dtypes

from contextlib import ExitStack

import concourse.bass as bass
import concourse.mybir as mybir
from concourse import bacc, tile
from concourse.bass import ds, ts
from concourse.bass_utils import run_bass_kernel_spmd
from concourse.tile_rust import add_dep_helper

BF16 = mybir.dt.bfloat16
F32 = mybir.dt.float32
AFT = mybir.ActivationFunctionType
ALU = mybir.AluOpType

N_CORES = 8
BATCH = 16384
B = BATCH // N_CORES  # per-core batch shard (2048)
H = 1024  # hidden == input size
KC = H // 128  # contraction chunks (8)
JT = H // 128  # output row tiles (8)
NB = 4  # moving-dim (batch) blocks per psum bank
NBS = B // NB  # 512 columns per matmul
GAMMA = 0.01

_nc_cache = {}


def _build(eps: float):
    """Build + compile the single-core Tile program (same graph on all cores)."""
    nc = bacc.Bacc("TRN2", target_bir_lowering=False, debug=False)

    xT_d = nc.dram_tensor("xT", [128, KC, B], BF16, kind="ExternalInput")
    hT_d = nc.dram_tensor("hT", [128, KC, B], BF16, kind="ExternalInput")
    w_d = {
        name: nc.dram_tensor(name, [JT, 128, KC, 128], BF16, kind="ExternalInput")
        for name in ["wzT", "uzT", "wrT", "urT", "vhT", "aT"]
    }
    bias_d = nc.dram_tensor("biases", [128, 24], F32, kind="ExternalInput")
    out_d = nc.dram_tensor("out", [H, B], F32, kind="ExternalOutput")

    with tile.TileContext(nc) as tc, ExitStack() as ctx:
        singles = ctx.enter_context(tc.tile_pool(name="singles", bufs=1))
        wpool = ctx.enter_context(tc.tile_pool(name="wpool", bufs=10))
        psum = ctx.enter_context(tc.tile_pool(name="psum", bufs=8, space="PSUM"))
        actp = ctx.enter_context(tc.tile_pool(name="actp", bufs=6))
        tmpp = ctx.enter_context(tc.tile_pool(name="tmpp", bufs=4))
        outp = ctx.enter_context(tc.tile_pool(name="outp", bufs=3))

        xT = singles.tile([128, KC, B], BF16)
        hTb = singles.tile([128, KC, B], BF16)
        rhT = singles.tile([128, KC, B], BF16)
        bias_sb = singles.tile([128, 24], F32)

        def load_w(name, jt):
            w = wpool.tile([128, KC, 128], BF16, tag="w")
            nc.sync.dma_start(out=w[:], in_=w_d[name][jt])
            return w

        # Weights for the first row-block go out first so the PE isn't stuck
        # behind the 8MB x/h stream on the DMA queues.
        wr0 = load_w("wrT", 0)
        ur0 = load_w("urT", 0)
        for c in range(KC):
            nc.sync.dma_start(out=xT[:, c, :], in_=xT_d[:, c, :])
            nc.sync.dma_start(out=hTb[:, c, :], in_=hT_d[:, c, :])
        nc.sync.dma_start(out=bias_sb[:], in_=bias_d[:])

        # All PE matmuls are chained in program order (ordering-only deps) so
        # that groups of matmuls sharing a stationary operand stay contiguous:
        # followers in each group skip their LDWEIGHTS (ldweights=False) and
        # reuse the weights already in the array.
        prev_mm = [None]

        def mm(psum_ap, w_ap, rhs_ap, start, stop, reload_w):
            bi = nc.tensor.matmul(psum_ap, w_ap, rhs_ap, start=start, stop=stop)
            if not reload_w:
                bi.ins.ldweights = False
            if prev_mm[0] is not None:
                add_dep_helper(bi.ins, prev_mm[0], False, "pe-order")
            prev_mm[0] = bi.ins
            return bi

        def gemm_pair(psums, wA, rhsA, wB, rhsB):
            # psums[nb] += wA[:,k,:].T @ rhsA[:,k,nb] summed over k, then wB/rhsB
            for k in range(KC):
                for nb in range(NB):
                    mm(
                        psums[nb][:],
                        wA[:, k, :],
                        rhsA[:, k, ds(nb * NBS, NBS)],
                        start=(k == 0),
                        stop=False,
                        reload_w=(nb == 0),
                    )
            for k in range(KC):
                for nb in range(NB):
                    mm(
                        psums[nb][:],
                        wB[:, k, :],
                        rhsB[:, k, ds(nb * NBS, NBS)],
                        start=False,
                        stop=(k == KC - 1),
                        reload_w=(nb == 0),
                    )

        # ---- phase 1: r gate (hidden-major), rhT = sigmoid(...) * hT ----
        for jt in range(JT):
            if jt == 0:
                wr, ur = wr0, ur0
            else:
                wr = load_w("wrT", jt)
                ur = load_w("urT", jt)
            ps = [
                psum.tile([128, NBS], F32, tag="ps", name=f"ps_r{jt}_{i}")
                for i in range(NB)
            ]
            gemm_pair(ps, wr, xT, ur, hTb)
            for nb in range(NB):
                rt = actp.tile([128, NBS], BF16, tag="act")
                nc.scalar.activation(
                    rt[:], ps[nb][:], AFT.Sigmoid, bias=bias_sb[:, 8 + jt : 9 + jt]
                )
                nc.vector.tensor_mul(
                    rhT[:, jt, ds(nb * NBS, NBS)],
                    rt[:],
                    hTb[:, jt, ds(nb * NBS, NBS)],
                )

        # ---- phase 2: z gate + delta_h + residual, one jt row-block at a time ----
        for jt in range(JT):
            wz = load_w("wzT", jt)
            uz = load_w("uzT", jt)
            vh = load_w("vhT", jt)
            at = load_w("aT", jt)
            psz = [
                psum.tile([128, NBS], F32, tag="ps", name=f"ps_z{jt}_{i}")
                for i in range(NB)
            ]
            gemm_pair(psz, wz, xT, uz, hTb)
            psd = [
                psum.tile([128, NBS], F32, tag="ps", name=f"ps_d{jt}_{i}")
                for i in range(NB)
            ]
            gemm_pair(psd, vh, xT, at, rhT)
            ot = outp.tile([128, B], F32, tag="out")
            for nb in range(NB):
                zt = actp.tile([128, NBS], BF16, tag="act")
                nc.scalar.activation(
                    zt[:], psz[nb][:], AFT.Sigmoid, bias=bias_sb[:, jt : jt + 1]
                )
                dt_ = actp.tile([128, NBS], BF16, tag="act")
                nc.scalar.activation(
                    dt_[:], psd[nb][:], AFT.Tanh, bias=bias_sb[:, 16 + jt : 17 + jt]
                )
                zdh = tmpp.tile([128, NBS], F32, tag="zdh")
                nc.vector.tensor_mul(zdh[:], zt[:], dt_[:])
                # out = (z*dh) * eps + h
                nc.vector.scalar_tensor_tensor(
                    ot[:, ds(nb * NBS, NBS)],
                    zdh[:],
                    float(eps),
                    hTb[:, jt, ds(nb * NBS, NBS)],
                    op0=ALU.mult,
                    op1=ALU.add,
                )
                nc.sync.dma_start(
                    out=out_d[ts(jt, 128), ds(nb * NBS, NBS)],
                    in_=ot[:, ds(nb * NBS, NBS)],
                )

    nc.compile()
    return nc


def _get_nc(eps: float):
    key = float(eps)
    if key not in _nc_cache:
        _nc_cache[key] = _build(key)
    return _nc_cache[key]


def _block_weight(wT):
    # [1024, 1024] (contraction-major) -> [jt, p, c, j] st. blk[jt,p,c,j] = wT[c*128+p, jt*128+j]
    return np.ascontiguousarray(
        wT.reshape(KC, 128, JT, 128).transpose(2, 1, 0, 3)
    ).astype(ml_dtypes.bfloat16)


def _block_data(m):
    # per-core [B, 1024] -> [p, c, b] st. blk[p,c,b] = m[b, c*128+p]
    return np.ascontiguousarray(m.T.reshape(KC, 128, B).transpose(1, 0, 2))


def _prep_in_maps(x, h_prev, W_z, b_z, U_z, W_r, b_r, U_r, V_h, b_h, W_h):
    x16 = np.asarray(x, np.float32).astype(ml_dtypes.bfloat16)
    h16 = np.asarray(h_prev, np.float32).astype(ml_dtypes.bfloat16)

    A = W_h - W_h.T - GAMMA * np.eye(H, dtype=np.float32)
    shared = {
        "wzT": _block_weight(W_z.T),
        "uzT": _block_weight(U_z.T),
        "wrT": _block_weight(W_r.T),
        "urT": _block_weight(U_r.T),
        "vhT": _block_weight(V_h.T),
        "aT": _block_weight(A.T),
        "biases": np.ascontiguousarray(
            np.concatenate(
                [
                    b_z.reshape(JT, 128).T,
                    b_r.reshape(JT, 128).T,
                    b_h.reshape(JT, 128).T,
                ],
                axis=1,
            ).astype(np.float32)
        ),
    }
    in_maps = []
    for c in range(N_CORES):
        sl = slice(c * B, (c + 1) * B)
        in_maps.append(
            {"xT": _block_data(x16[sl]), "hT": _block_data(h16[sl]), **shared}
        )
    return in_maps


def run(inputs, trace=False):
    """Returns (full_output [16384,1024] f32, BassKernelResults)."""
    np_in = {k: np.asarray(v, np.float32) for k, v in inputs.items()}
    eps = float(np_in.pop("epsilon"))
    in_maps = _prep_in_maps(**np_in)
    nc = _get_nc(eps)
    res = run_bass_kernel_spmd(
        nc, in_maps, core_ids=list(range(N_CORES)), trace=trace
    )
    out = np.empty((BATCH, H), np.float32)
    for c in range(N_CORES):
        out[c * B : (c + 1) * B, :] = res.results[c]["out"].T
    return out, res


def kernel(**inputs) -> np.ndarray:
    out, _ = run(inputs, trace=False)
    return out


# revision 11
# speedup vs baseline: 1.0144x; 1.0036x over previous
"""AGRU cell (antisymmetric GRU) forward on 8 TRN2 NeuronCores.

Data-parallel: batch 16384 is sharded 2048 rows/core; the six 1024x1024
weight matrices are replicated. No cross-core communication.

Everything on-device is computed in "hidden-major" (transposed) layout:
    zT = sigmoid(Wz @ xT + Uz @ hT + bz)        [H, B]
    rT = sigmoid(Wr @ xT + Ur @ hT + br)
    rhT = rT * hT
    dhT = tanh(Vh @ xT + A @ rhT + bh)
    outT = hT + eps * zT * dhT
so every matmul has the (pre-transposed, host-prepared) weight tile as the
stationary operand and xT/hT/rhT as the moving operand, and nothing ever
needs an on-device transpose.  The host transposes each core's [1024, 2048]
result back when assembling the full output.

Matmuls run in bf16 (1 cycle/row on TRN2 vs 4 for fp32) with fp32 PSUM
accumulation; the final residual add is done in fp32.
"""

import sys

sys.path.insert(0, "/opt/trn_rl_repo")

import numpy as np
import ml_dtypes

from contextlib import ExitStack

import concourse.bass as bass
import concourse.mybir as mybir
from concourse import bacc, tile
from concourse.bass import ds, ts
from concourse.bass_utils import run_bass_kernel_spmd
from concourse.tile_rust import add_dep_helper

BF16 = mybir.dt.bfloat16
F32 = mybir.dt.float32
AFT = mybir.ActivationFunctionType
ALU = mybir.AluOpType

N_CORES = 8
BATCH = 16384
B = BATCH // N_CORES  # per-core batch shard (2048)
H = 1024  # hidden == input size
KC = H // 128  # contraction chunks (8)
JT = H // 128  # output row tiles (8)
NB = 4  # moving-dim (batch) blocks per psum bank
NBS = B // NB  # 512 columns per matmul
GAMMA = 0.01

_nc_cache = {}


def _build(eps: float):
    """Build + compile the single-core Tile program (same graph on all cores)."""
    nc = bacc.Bacc("TRN2", target_bir_lowering=False, debug=False)

    xT_d = nc.dram_tensor("xT", [128, KC, B], BF16, kind="ExternalInput")
    hT_d = nc.dram_tensor("hT", [128, KC, B], BF16, kind="ExternalInput")
    w_d = {
        name: nc.dram_tensor(name, [JT, 128, KC, 128], BF16, kind="ExternalInput")
        for name in ["wzT", "uzT", "wrT", "urT", "vhT", "aT"]
    }
    bias_d = nc.dram_tensor("biases", [128, 24], F32, kind="ExternalInput")
    out_d = nc.dram_tensor("out", [H, B], F32, kind="ExternalOutput")

    with tile.TileContext(nc) as tc, ExitStack() as ctx:
        singles = ctx.enter_context(tc.tile_pool(name="singles", bufs=1))
        wpool = ctx.enter_context(tc.tile_pool(name="wpool", bufs=10))
        psum = ctx.enter_context(tc.tile_pool(name="psum", bufs=8, space="PSUM"))
        actp = ctx.enter_context(tc.tile_pool(name="actp", bufs=6))
        tmpp = ctx.enter_context(tc.tile_pool(name="tmpp", bufs=4))
        outp = ctx.enter_context(tc.tile_pool(name="outp", bufs=3))

        xT = singles.tile([128, KC, B], BF16)
        hTb = singles.tile([128, KC, B], BF16)
        rhT = singles.tile([128, KC, B], BF16)
        bias_sb = singles.tile([128, 24], F32)

        def load_w(name, jt):
            w = wpool.tile([128, KC, 128], BF16, tag="w")
            nc.sync.dma_start(out=w[:], in_=w_d[name][jt])
            return w

        # Weights for the first row-block go out first so the PE isn't stuck
        # behind the 8MB x/h stream on the DMA queues.
        wr0 = load_w("wrT", 0)
        ur0 = load_w("urT", 0)
        for c in range(KC):
            nc.sync.dma_start(out=xT[:, c, :], in_=xT_d[:, c, :])
            nc.sync.dma_start(out=hTb[:, c, :], in_=hT_d[:, c, :])
        nc.sync.dma_start(out=bias_sb[:], in_=bias_d[:])

        # All PE matmuls are chained in program order (ordering-only deps) so
        # that groups of matmuls sharing a stationary operand stay contiguous:
        # followers in each group skip their LDWEIGHTS (ldweights=False) and
        # reuse the weights already in the array.
        prev_mm = [None]

        def mm(psum_ap, w_ap, rhs_ap, start, stop, reload_w):
            bi = nc.tensor.matmul(psum_ap, w_ap, rhs_ap, start=start, stop=stop)
            if not reload_w:
                bi.ins.ldweights = False
            if prev_mm[0] is not None:
                add_dep_helper(bi.ins, prev_mm[0], False, "pe-order")
            prev_mm[0] = bi.ins
            return bi

        def gemm_pair(psums, wA, rhsA, wB, rhsB):
            # psums[nb] += wA[:,k,:].T @ rhsA[:,k,nb] summed over k, then wB/rhsB
            for k in range(KC):
                for nb in range(NB):
                    mm(
                        psums[nb][:],
                        wA[:, k, :],
                        rhsA[:, k, ds(nb * NBS, NBS)],
                        start=(k == 0),
                        stop=False,
                        reload_w=(nb == 0),
                    )
            for k in range(KC):
                for nb in range(NB):
                    mm(
                        psums[nb][:],
                        wB[:, k, :],
                        rhsB[:, k, ds(nb * NBS, NBS)],
                        start=False,
                        stop=(k == KC - 1),
                        reload_w=(nb == 0),
                    )

        # ---- phase 1: r gate (hidden-major), rhT = sigmoid(...) * hT ----
        for jt in range(JT):
            if jt == 0:
                wr, ur = wr0, ur0
            else:
                wr = load_w("wrT", jt)
                ur = load_w("urT", jt)
            ps = [
                psum.tile([128, NBS], F32, tag="ps", name=f"ps_r{jt}_{i}")
                for i in range(NB)
            ]
            gemm_pair(ps, wr, xT, ur, hTb)
            for nb in range(NB):
                rt = actp.tile([128, NBS], BF16, tag="act")
                nc.scalar.activation(
                    rt[:], ps[nb][:], AFT.Sigmoid, bias=bias_sb[:, 8 + jt : 9 + jt]
                )
                nc.vector.tensor_mul(
                    rhT[:, jt, ds(nb * NBS, NBS)],
                    rt[:],
                    hTb[:, jt, ds(nb * NBS, NBS)],
                )

        # ---- phase 2: z gate + delta_h + residual, one jt row-block at a time ----
        for jt in range(JT):
            wz = load_w("wzT", jt)
            uz = load_w("uzT", jt)
            vh = load_w("vhT", jt)
            at = load_w("aT", jt)
            psz = [
                psum.tile([128, NBS], F32, tag="ps", name=f"ps_z{jt}_{i}")
                for i in range(NB)
            ]
            gemm_pair(psz, wz, xT, uz, hTb)
            psd = [
                psum.tile([128, NBS], F32, tag="ps", name=f"ps_d{jt}_{i}")
                for i in range(NB)
            ]
            gemm_pair(psd, vh, xT, at, rhT)
            ot = outp.tile([128, B], F32, tag="out")
            for nb in range(NB):
                zt = actp.tile([128, NBS], BF16, tag="act")
                nc.scalar.activation(
                    zt[:], psz[nb][:], AFT.Sigmoid, bias=bias_sb[:, jt : jt + 1]
                )
                dt_ = actp.tile([128, NBS], BF16, tag="act")
                nc.scalar.activation(
                    dt_[:], psd[nb][:], AFT.Tanh, bias=bias_sb[:, 16 + jt : 17 + jt]
                )
                zdh = tmpp.tile([128, NBS], F32, tag="zdh")
                nc.vector.tensor_mul(zdh[:], zt[:], dt_[:])
                # out = (z*dh) * eps + h
                nc.vector.scalar_tensor_tensor(
                    ot[:, ds(nb * NBS, NBS)],
                    zdh[:],
                    float(eps),
                    hTb[:, jt, ds(nb * NBS, NBS)],
                    op0=ALU.mult,
                    op1=ALU.add,
                )
                nc.sync.dma_start(
                    out=out_d[ts(jt, 128), ds(nb * NBS, NBS)],
                    in_=ot[:, ds(nb * NBS, NBS)],
                )

    _dedupe_ldweights(nc)
    nc.compile()
    return nc


def _dedupe_ldweights(nc):
    """Drop back-to-back InstLdweights with identical weight APs.

    Tile legalization splits every bf16 matmul into LDWEIGHTS+MATMUL even when
    consecutive matmuls share the stationary operand. The PE executes its
    stream in order, so a repeated load of the same weights is pure overhead
    (~128 cycles per 512-cycle matmul). Only drops loads that carry no
    semaphore waits/updates; the explicit pe-order dep chain built in _build
    guarantees groups sharing weights are contiguous in the stream.
    """
    removed = 0
    for blk in nc.m.functions[0].blocks:
        new = []
        last_key = None
        for i in blk.instructions:
            if i.engine == mybir.EngineType.PE:
                if isinstance(i, mybir.InstLdweights):
                    si = i.sync_info
                    clean = si is None or (not si.on_wait and not si.on_update)
                    key = str(i.ins[0])
                    if clean and key == last_key:
                        removed += 1
                        continue
                    last_key = key
                elif not isinstance(i, mybir.InstMatmult):
                    last_key = None
            new.append(i)
        blk.instructions[:] = new
    return removed


def _get_nc(eps: float):
    key = float(eps)
    if key not in _nc_cache:
        _nc_cache[key] = _build(key)
    return _nc_cache[key]


def _block_weight(wT):
    # [1024, 1024] (contraction-major) -> [jt, p, c, j] st. blk[jt,p,c,j] = wT[c*128+p, jt*128+j]
    return np.ascontiguousarray(
        wT.reshape(KC, 128, JT, 128).transpose(2, 1, 0, 3)
    ).astype(ml_dtypes.bfloat16)


def _block_data(m):
    # per-core [B, 1024] -> [p, c, b] st. blk[p,c,b] = m[b, c*128+p]
    return np.ascontiguousarray(m.T.reshape(KC, 128, B).transpose(1, 0, 2))


def _prep_in_maps(x, h_prev, W_z, b_z, U_z, W_r, b_r, U_r, V_h, b_h, W_h):
    x16 = np.asarray(x, np.float32).astype(ml_dtypes.bfloat16)
    h16 = np.asarray(h_prev, np.float32).astype(ml_dtypes.bfloat16)

    A = W_h - W_h.T - GAMMA * np.eye(H, dtype=np.float32)
    shared = {
        "wzT": _block_weight(W_z.T),
        "uzT": _block_weight(U_z.T),
        "wrT": _block_weight(W_r.T),
        "urT": _block_weight(U_r.T),
        "vhT": _block_weight(V_h.T),
        "aT": _block_weight(A.T),
        "biases": np.ascontiguousarray(
            np.concatenate(
                [
                    b_z.reshape(JT, 128).T,
                    b_r.reshape(JT, 128).T,
                    b_h.reshape(JT, 128).T,
                ],
                axis=1,
            ).astype(np.float32)
        ),
    }
    in_maps = []
    for c in range(N_CORES):
        sl = slice(c * B, (c + 1) * B)
        in_maps.append(
            {"xT": _block_data(x16[sl]), "hT": _block_data(h16[sl]), **shared}
        )
    return in_maps


def run(inputs, trace=False):
    """Returns (full_output [16384,1024] f32, BassKernelResults)."""
    np_in = {k: np.asarray(v, np.float32) for k, v in inputs.items()}
    eps = float(np_in.pop("epsilon"))
    in_maps = _prep_in_maps(**np_in)
    nc = _get_nc(eps)
    res = run_bass_kernel_spmd(
        nc, in_maps, core_ids=list(range(N_CORES)), trace=trace
    )
    out = np.empty((BATCH, H), np.float32)
    for c in range(N_CORES):
        out[c * B : (c + 1) * B, :] = res.results[c]["out"].T
    return out, res


def kernel(**inputs) -> np.ndarray:
    out, _ = run(inputs, trace=False)
    return out


# revision 19
# speedup vs baseline: 1.5897x; 1.5671x over previous
"""AGRU cell (antisymmetric GRU) forward on 8 TRN2 NeuronCores.

Data-parallel: batch 16384 is sharded 2048 rows/core; the six 1024x1024
weight matrices are replicated. No cross-core communication.

Everything on-device is computed in "hidden-major" (transposed) layout:
    zT = sigmoid(Wz @ xT + Uz @ hT + bz)        [H, B]
    rT = sigmoid(Wr @ xT + Ur @ hT + br)
    rhT = rT * hT
    dhT = tanh(Vh @ xT + A @ rhT + bh)
    outT = hT + eps * zT * dhT
so every matmul has the (pre-transposed, host-prepared) weight tile as the
stationary operand and xT/hT/rhT as the moving operand, and nothing ever
needs an on-device transpose.  The host transposes each core's [1024, 2048]
result back when assembling the full output.

Matmuls run in bf16 (1 cycle/row on TRN2 vs 4 for fp32) with fp32 PSUM
accumulation; the final residual add is done in fp32.
"""

import sys

sys.path.insert(0, "/opt/trn_rl_repo")

import numpy as np
import ml_dtypes

from contextlib import ExitStack

import concourse.bass as bass
import concourse.mybir as mybir
from concourse import bacc, tile
from concourse.bass import ds, ts
from concourse.bass_utils import run_bass_kernel_spmd
from concourse.tile_rust import add_dep_helper

BF16 = mybir.dt.bfloat16
FP8 = mybir.dt.float8e4
F32 = mybir.dt.float32
AFT = mybir.ActivationFunctionType
ALU = mybir.AluOpType
DR = mybir.MatmulPerfMode.DoubleRow

# fp8 pre-scaling for the sigmoid-gate GEMMs (z, r): data*16, weights*256,
# compensated by activation scale 1/(16*256).
SCALE_X = 16.0
SCALE_W = 256.0
INV_SCALE = 1.0 / (SCALE_X * SCALE_W)

N_CORES = 8
BATCH = 16384
B = BATCH // N_CORES  # per-core batch shard (2048)
H = 1024  # hidden == input size
KC = H // 128  # contraction chunks (8)
JT = H // 128  # output row tiles (8)
NB = 4  # moving-dim (batch) blocks per psum bank
NBS = B // NB  # 512 columns per matmul
GAMMA = 0.01

_nc_cache = {}


def _build(eps: float):
    """Build + compile the single-core Tile program (same graph on all cores)."""
    nc = bacc.Bacc("TRN2", target_bir_lowering=False, debug=False)

    xT_d = nc.dram_tensor("xT", [128, KC, B], BF16, kind="ExternalInput")
    hT_d = nc.dram_tensor("hT", [128, KC, B], BF16, kind="ExternalInput")
    xT8_d = nc.dram_tensor("xT8", [128, KC, B], FP8, kind="ExternalInput")
    hT8_d = nc.dram_tensor("hT8", [128, KC, B], FP8, kind="ExternalInput")
    w_d = {
        name: nc.dram_tensor(name, [JT, 128, KC, 128], BF16, kind="ExternalInput")
        for name in ["vhT", "aT"]
    }
    w8_d = {
        name: nc.dram_tensor(name, [JT, 128, KC, 128], FP8, kind="ExternalInput")
        for name in ["wz8", "uz8", "wr8", "ur8"]
    }
    bias_d = nc.dram_tensor("biases", [128, 24], F32, kind="ExternalInput")
    out_d = nc.dram_tensor("out", [H, B], F32, kind="ExternalOutput")

    with tile.TileContext(nc) as tc, ExitStack() as ctx:
        singles = ctx.enter_context(tc.tile_pool(name="singles", bufs=1))
        wpool = ctx.enter_context(tc.tile_pool(name="wpool", bufs=8))
        psum = ctx.enter_context(tc.tile_pool(name="psum", bufs=8, space="PSUM"))
        actp = ctx.enter_context(tc.tile_pool(name="actp", bufs=6))
        tmpp = ctx.enter_context(tc.tile_pool(name="tmpp", bufs=4))
        outp = ctx.enter_context(tc.tile_pool(name="outp", bufs=2))

        xT = singles.tile([128, KC, B], BF16)
        hTb = singles.tile([128, KC, B], BF16)
        xT8 = singles.tile([128, KC, B], FP8)
        hT8 = singles.tile([128, KC, B], FP8)
        rhT = singles.tile([128, KC, B], BF16)
        bias_sb = singles.tile([128, 24], F32)

        def load_w(name, jt):
            fp8 = name in w8_d
            w = wpool.tile([128, KC, 128], FP8 if fp8 else BF16, tag="w")
            nc.sync.dma_start(out=w[:], in_=(w8_d[name] if fp8 else w_d[name])[jt])
            return w

        # Weights for the first row-block go out first so the PE isn't stuck
        # behind the x/h stream on the DMA queues.
        wr0 = load_w("wr8", 0)
        ur0 = load_w("ur8", 0)
        for c in range(KC):
            nc.sync.dma_start(out=xT8[:, c, :], in_=xT8_d[:, c, :])
            nc.sync.dma_start(out=hT8[:, c, :], in_=hT8_d[:, c, :])
        for c in range(KC):
            nc.sync.dma_start(out=xT[:, c, :], in_=xT_d[:, c, :])
            nc.sync.dma_start(out=hTb[:, c, :], in_=hT_d[:, c, :])
        nc.sync.dma_start(out=bias_sb[:], in_=bias_d[:])

        # All PE matmuls are chained in program order (ordering-only deps) so
        # that groups of matmuls sharing a stationary operand stay contiguous:
        # followers in each group skip their LDWEIGHTS (ldweights=False) and
        # reuse the weights already in the array.
        prev_mm = [None]

        def mm(psum_ap, w_ap, rhs_ap, start, stop, reload_w, perf_mode=None):
            bi = nc.tensor.matmul(
                psum_ap, w_ap, rhs_ap, start=start, stop=stop, perf_mode=perf_mode
            )
            if not reload_w:
                bi.ins.ldweights = False
            if prev_mm[0] is not None:
                add_dep_helper(bi.ins, prev_mm[0], False, "pe-order")
            prev_mm[0] = bi.ins
            return bi

        def gemm_pair(psums, wA, rhsA, wB, rhsB):
            # psums[nb] += wA[:,k,:].T @ rhsA[:,k,nb] summed over k, then wB/rhsB
            for k in range(KC):
                for nb in range(NB):
                    mm(
                        psums[nb][:],
                        wA[:, k, :],
                        rhsA[:, k, ds(nb * NBS, NBS)],
                        start=(k == 0),
                        stop=False,
                        reload_w=(nb == 0),
                    )
            for k in range(KC):
                for nb in range(NB):
                    mm(
                        psums[nb][:],
                        wB[:, k, :],
                        rhsB[:, k, ds(nb * NBS, NBS)],
                        start=False,
                        stop=(k == KC - 1),
                        reload_w=(nb == 0),
                    )

        def gemm_pair_fp8(psums, wA, rhsA, wB, rhsB):
            # fp8 DoubleRow: each matmul covers two 128-row contraction chunks
            for k in range(0, KC, 2):
                for nb in range(NB):
                    mm(
                        psums[nb][:],
                        wA[:, k : k + 2, :],
                        rhsA[:, k : k + 2, ds(nb * NBS, NBS)],
                        start=(k == 0),
                        stop=False,
                        reload_w=(nb == 0),
                        perf_mode=DR,
                    )
            for k in range(0, KC, 2):
                for nb in range(NB):
                    mm(
                        psums[nb][:],
                        wB[:, k : k + 2, :],
                        rhsB[:, k : k + 2, ds(nb * NBS, NBS)],
                        start=False,
                        stop=(k == KC - 2),
                        reload_w=(nb == 0),
                        perf_mode=DR,
                    )

        # ---- phase 1: r gate (hidden-major, fp8), rhT = sigmoid(...) * hT ----
        for jt in range(JT):
            if jt == 0:
                wr, ur = wr0, ur0
            else:
                wr = load_w("wr8", jt)
                ur = load_w("ur8", jt)
            ps = [
                psum.tile([128, NBS], F32, tag="ps", name=f"ps_r{jt}_{i}")
                for i in range(NB)
            ]
            gemm_pair_fp8(ps, wr, xT8, ur, hT8)
            for nb in range(NB):
                rt = actp.tile([128, NBS], BF16, tag="act")
                nc.scalar.activation(
                    rt[:],
                    ps[nb][:],
                    AFT.Sigmoid,
                    bias=bias_sb[:, 8 + jt : 9 + jt],
                    scale=INV_SCALE,
                )
                nc.vector.tensor_mul(
                    rhT[:, jt, ds(nb * NBS, NBS)],
                    rt[:],
                    hTb[:, jt, ds(nb * NBS, NBS)],
                )

        # ---- phase 2: z gate (fp8) + delta_h (bf16) + residual, per jt ----
        for jt in range(JT):
            wz = load_w("wz8", jt)
            uz = load_w("uz8", jt)
            vh = load_w("vhT", jt)
            at = load_w("aT", jt)
            psz = [
                psum.tile([128, NBS], F32, tag="ps", name=f"ps_z{jt}_{i}")
                for i in range(NB)
            ]
            gemm_pair_fp8(psz, wz, xT8, uz, hT8)
            psd = [
                psum.tile([128, NBS], F32, tag="ps", name=f"ps_d{jt}_{i}")
                for i in range(NB)
            ]
            gemm_pair(psd, vh, xT, at, rhT)
            ot = outp.tile([128, B], F32, tag="out")
            for nb in range(NB):
                zt = actp.tile([128, NBS], BF16, tag="act")
                nc.scalar.activation(
                    zt[:],
                    psz[nb][:],
                    AFT.Sigmoid,
                    bias=bias_sb[:, jt : jt + 1],
                    scale=INV_SCALE,
                )
                dt_ = actp.tile([128, NBS], BF16, tag="act")
                nc.scalar.activation(
                    dt_[:], psd[nb][:], AFT.Tanh, bias=bias_sb[:, 16 + jt : 17 + jt]
                )
                zdh = tmpp.tile([128, NBS], F32, tag="zdh")
                nc.vector.tensor_mul(zdh[:], zt[:], dt_[:])
                # out = (z*dh) * eps + h
                nc.vector.scalar_tensor_tensor(
                    ot[:, ds(nb * NBS, NBS)],
                    zdh[:],
                    float(eps),
                    hTb[:, jt, ds(nb * NBS, NBS)],
                    op0=ALU.mult,
                    op1=ALU.add,
                )
                nc.sync.dma_start(
                    out=out_d[ts(jt, 128), ds(nb * NBS, NBS)],
                    in_=ot[:, ds(nb * NBS, NBS)],
                )

    _dedupe_ldweights(nc)
    nc.compile()
    return nc


def _dedupe_ldweights(nc):
    """Drop back-to-back InstLdweights with identical weight APs.

    Tile legalization splits every bf16 matmul into LDWEIGHTS+MATMUL even when
    consecutive matmuls share the stationary operand. The PE executes its
    stream in order, so a repeated load of the same weights is pure overhead
    (~128 cycles per 512-cycle matmul). Only drops loads that carry no
    semaphore waits/updates; the explicit pe-order dep chain built in _build
    guarantees groups sharing weights are contiguous in the stream.
    """
    removed = 0
    for blk in nc.m.functions[0].blocks:
        new = []
        last_key = None
        for i in blk.instructions:
            if i.engine == mybir.EngineType.PE:
                if isinstance(i, mybir.InstLdweights):
                    si = i.sync_info
                    clean = si is None or (not si.on_wait and not si.on_update)
                    key = str(i.ins[0])
                    if clean and key == last_key:
                        removed += 1
                        continue
                    last_key = key
                elif not isinstance(i, mybir.InstMatmult):
                    last_key = None
            new.append(i)
        blk.instructions[:] = new
    return removed


def _get_nc(eps: float):
    key = float(eps)
    if key not in _nc_cache:
        _nc_cache[key] = _build(key)
    return _nc_cache[key]


def _block_weight(wT, dtype, scale=1.0):
    # [1024, 1024] (contraction-major) -> [jt, p, c, j] st. blk[jt,p,c,j] = wT[c*128+p, jt*128+j]
    blk = wT.reshape(KC, 128, JT, 128).transpose(2, 1, 0, 3)
    if scale != 1.0:
        blk = blk * scale
    return np.ascontiguousarray(blk).astype(dtype)


def _block_data(m):
    # per-core [B, 1024] -> [p, c, b] st. blk[p,c,b] = m[b, c*128+p]
    return np.ascontiguousarray(m.T.reshape(KC, 128, B).transpose(1, 0, 2))


def _prep_in_maps(x, h_prev, W_z, b_z, U_z, W_r, b_r, U_r, V_h, b_h, W_h):
    BF = ml_dtypes.bfloat16
    F8 = ml_dtypes.float8_e4m3
    x16 = np.asarray(x, np.float32).astype(BF)
    h16 = np.asarray(h_prev, np.float32).astype(BF)
    x8 = (np.asarray(x, np.float32) * SCALE_X).astype(F8)
    h8 = (np.asarray(h_prev, np.float32) * SCALE_X).astype(F8)

    A = W_h - W_h.T - GAMMA * np.eye(H, dtype=np.float32)
    shared = {
        "wz8": _block_weight(W_z.T, F8, SCALE_W),
        "uz8": _block_weight(U_z.T, F8, SCALE_W),
        "wr8": _block_weight(W_r.T, F8, SCALE_W),
        "ur8": _block_weight(U_r.T, F8, SCALE_W),
        "vhT": _block_weight(V_h.T, BF),
        "aT": _block_weight(A.T, BF),
        "biases": np.ascontiguousarray(
            np.concatenate(
                [
                    b_z.reshape(JT, 128).T,
                    b_r.reshape(JT, 128).T,
                    b_h.reshape(JT, 128).T,
                ],
                axis=1,
            ).astype(np.float32)
        ),
    }
    in_maps = []
    for c in range(N_CORES):
        sl = slice(c * B, (c + 1) * B)
        in_maps.append(
            {
                "xT": _block_data(x16[sl]),
                "hT": _block_data(h16[sl]),
                "xT8": _block_data(x8[sl]),
                "hT8": _block_data(h8[sl]),
                **shared,
            }
        )
    return in_maps


def run(inputs, trace=False):
    """Returns (full_output [16384,1024] f32, BassKernelResults)."""
    np_in = {k: np.asarray(v, np.float32) for k, v in inputs.items()}
    eps = float(np_in.pop("epsilon"))
    in_maps = _prep_in_maps(**np_in)
    nc = _get_nc(eps)
    res = run_bass_kernel_spmd(
        nc, in_maps, core_ids=list(range(N_CORES)), trace=trace
    )
    out = np.empty((BATCH, H), np.float32)
    for c in range(N_CORES):
        out[c * B : (c + 1) * B, :] = res.results[c]["out"].T
    return out, res


def kernel(**inputs) -> np.ndarray:
    out, _ = run(inputs, trace=False)
    return out


# revision 21
# speedup vs baseline: 1.6139x; 1.0153x over previous
"""AGRU cell (antisymmetric GRU) forward on 8 TRN2 NeuronCores.

Data-parallel: batch 16384 is sharded 2048 rows/core; the six 1024x1024
weight matrices are replicated. No cross-core communication.

Everything on-device is computed in "hidden-major" (transposed) layout:
    zT = sigmoid(Wz @ xT + Uz @ hT + bz)        [H, B]
    rT = sigmoid(Wr @ xT + Ur @ hT + br)
    rhT = rT * hT
    dhT = tanh(Vh @ xT + A @ rhT + bh)
    outT = hT + eps * zT * dhT
so every matmul has the (pre-transposed, host-prepared) weight tile as the
stationary operand and xT/hT/rhT as the moving operand, and nothing ever
needs an on-device transpose.  The host transposes each core's [1024, 2048]
result back when assembling the full output.

Matmuls run in bf16 (1 cycle/row on TRN2 vs 4 for fp32) with fp32 PSUM
accumulation; the final residual add is done in fp32.
"""

import sys

sys.path.insert(0, "/opt/trn_rl_repo")

import numpy as np
import ml_dtypes

from contextlib import ExitStack

import concourse.bass as bass
import concourse.mybir as mybir
from concourse import bacc, tile
from concourse.bass import ds, ts
from concourse.bass_utils import run_bass_kernel_spmd
from concourse.tile_rust import add_dep_helper

BF16 = mybir.dt.bfloat16
FP8 = mybir.dt.float8e4
F32 = mybir.dt.float32
AFT = mybir.ActivationFunctionType
ALU = mybir.AluOpType
DR = mybir.MatmulPerfMode.DoubleRow

# fp8 pre-scaling for the sigmoid-gate GEMMs (z, r): data*16, weights*256,
# compensated by activation scale 1/(16*256).
SCALE_X = 16.0
SCALE_W = 256.0
INV_SCALE = 1.0 / (SCALE_X * SCALE_W)

N_CORES = 8
BATCH = 16384
B = BATCH // N_CORES  # per-core batch shard (2048)
H = 1024  # hidden == input size
KC = H // 128  # contraction chunks (8)
JT = H // 128  # output row tiles (8)
NB = 4  # moving-dim (batch) blocks per psum bank
NBS = B // NB  # 512 columns per matmul
GAMMA = 0.01

_nc_cache = {}


def _build(eps: float):
    """Build + compile the single-core Tile program (same graph on all cores)."""
    nc = bacc.Bacc("TRN2", target_bir_lowering=False, debug=False)

    xT_d = nc.dram_tensor("xT", [128, KC, B], BF16, kind="ExternalInput")
    hT_d = nc.dram_tensor("hT", [128, KC, B], BF16, kind="ExternalInput")
    xT8_d = nc.dram_tensor("xT8", [128, KC, B], FP8, kind="ExternalInput")
    hT8_d = nc.dram_tensor("hT8", [128, KC, B], FP8, kind="ExternalInput")
    w_d = {
        name: nc.dram_tensor(name, [JT, 128, KC, 128], BF16, kind="ExternalInput")
        for name in ["vhT", "aT"]
    }
    w8_d = {
        name: nc.dram_tensor(name, [JT, 128, KC, 128], FP8, kind="ExternalInput")
        for name in ["wz8", "uz8", "wr8", "ur8"]
    }
    bias_d = nc.dram_tensor("biases", [128, 24], F32, kind="ExternalInput")
    out_d = nc.dram_tensor("out", [H, B], F32, kind="ExternalOutput")

    with tile.TileContext(nc) as tc, ExitStack() as ctx:
        singles = ctx.enter_context(tc.tile_pool(name="singles", bufs=1))
        wpool = ctx.enter_context(tc.tile_pool(name="wpool", bufs=8))
        psum = ctx.enter_context(tc.tile_pool(name="psum", bufs=8, space="PSUM"))
        actp = ctx.enter_context(tc.tile_pool(name="actp", bufs=10))
        tmpp = ctx.enter_context(tc.tile_pool(name="tmpp", bufs=4))
        outp = ctx.enter_context(tc.tile_pool(name="outp", bufs=2))

        xT = singles.tile([128, KC, B], BF16)
        hTb = singles.tile([128, KC, B], BF16)
        xT8 = singles.tile([128, KC, B], FP8)
        hT8 = singles.tile([128, KC, B], FP8)
        rhT = singles.tile([128, KC, B], BF16)
        bias_sb = singles.tile([128, 24], F32)

        def load_w(name, jt):
            fp8 = name in w8_d
            w = wpool.tile([128, KC, 128], FP8 if fp8 else BF16, tag="w")
            nc.sync.dma_start(out=w[:], in_=(w8_d[name] if fp8 else w_d[name])[jt])
            return w

        # Weights for the first row-block go out first so the PE isn't stuck
        # behind the x/h stream on the DMA queues.
        wr0 = load_w("wr8", 0)
        ur0 = load_w("ur8", 0)
        for c in range(KC):
            nc.sync.dma_start(out=xT8[:, c, :], in_=xT8_d[:, c, :])
            nc.sync.dma_start(out=hT8[:, c, :], in_=hT8_d[:, c, :])
        nc.sync.dma_start(out=bias_sb[:], in_=bias_d[:])
        # bf16 h before bf16 x: phase-1's rh-multiplies consume hTb chunks
        # almost immediately, while bf16 x is only needed in phase 2.
        for c in range(KC):
            nc.sync.dma_start(out=hTb[:, c, :], in_=hT_d[:, c, :])
        for c in range(KC):
            nc.sync.dma_start(out=xT[:, c, :], in_=xT_d[:, c, :])

        # All PE matmuls are chained in program order (ordering-only deps) so
        # that groups of matmuls sharing a stationary operand stay contiguous:
        # followers in each group skip their LDWEIGHTS (ldweights=False) and
        # reuse the weights already in the array.
        prev_mm = [None]

        def mm(psum_ap, w_ap, rhs_ap, start, stop, reload_w, perf_mode=None):
            bi = nc.tensor.matmul(
                psum_ap, w_ap, rhs_ap, start=start, stop=stop, perf_mode=perf_mode
            )
            if not reload_w:
                bi.ins.ldweights = False
            if prev_mm[0] is not None:
                add_dep_helper(bi.ins, prev_mm[0], False, "pe-order")
            prev_mm[0] = bi.ins
            return bi

        def gemm_pair(psums, wA, rhsA, wB, rhsB):
            # psums[nb] += wA[:,k,:].T @ rhsA[:,k,nb] summed over k, then wB/rhsB
            for k in range(KC):
                for nb in range(NB):
                    mm(
                        psums[nb][:],
                        wA[:, k, :],
                        rhsA[:, k, ds(nb * NBS, NBS)],
                        start=(k == 0),
                        stop=False,
                        reload_w=(nb == 0),
                    )
            for k in range(KC):
                for nb in range(NB):
                    mm(
                        psums[nb][:],
                        wB[:, k, :],
                        rhsB[:, k, ds(nb * NBS, NBS)],
                        start=False,
                        stop=(k == KC - 1),
                        reload_w=(nb == 0),
                    )

        def gemm_pair_fp8(psums, wA, rhsA, wB, rhsB):
            # fp8 DoubleRow: each matmul covers two 128-row contraction chunks
            for k in range(0, KC, 2):
                for nb in range(NB):
                    mm(
                        psums[nb][:],
                        wA[:, k : k + 2, :],
                        rhsA[:, k : k + 2, ds(nb * NBS, NBS)],
                        start=(k == 0),
                        stop=False,
                        reload_w=(nb == 0),
                        perf_mode=DR,
                    )
            for k in range(0, KC, 2):
                for nb in range(NB):
                    mm(
                        psums[nb][:],
                        wB[:, k : k + 2, :],
                        rhsB[:, k : k + 2, ds(nb * NBS, NBS)],
                        start=False,
                        stop=(k == KC - 2),
                        reload_w=(nb == 0),
                        perf_mode=DR,
                    )

        # ---- phase 1: r gate (hidden-major, fp8), rhT = sigmoid(...) * hT ----
        for jt in range(JT):
            if jt == 0:
                wr, ur = wr0, ur0
            else:
                wr = load_w("wr8", jt)
                ur = load_w("ur8", jt)
            ps = [
                psum.tile([128, NBS], F32, tag="ps", name=f"ps_r{jt}_{i}")
                for i in range(NB)
            ]
            gemm_pair_fp8(ps, wr, xT8, ur, hT8)
            for nb in range(NB):
                rt = actp.tile([128, NBS], BF16, tag="act")
                nc.scalar.activation(
                    rt[:],
                    ps[nb][:],
                    AFT.Sigmoid,
                    bias=bias_sb[:, 8 + jt : 9 + jt],
                    scale=INV_SCALE,
                )
                nc.vector.tensor_mul(
                    rhT[:, jt, ds(nb * NBS, NBS)],
                    rt[:],
                    hTb[:, jt, ds(nb * NBS, NBS)],
                )

        # ---- phase 2: z gate (fp8) + delta_h (bf16) + residual, per jt ----
        for jt in range(JT):
            wz = load_w("wz8", jt)
            uz = load_w("uz8", jt)
            vh = load_w("vhT", jt)
            at = load_w("aT", jt)
            psz = [
                psum.tile([128, NBS], F32, tag="ps", name=f"ps_z{jt}_{i}")
                for i in range(NB)
            ]
            gemm_pair_fp8(psz, wz, xT8, uz, hT8)
            psd = [
                psum.tile([128, NBS], F32, tag="ps", name=f"ps_d{jt}_{i}")
                for i in range(NB)
            ]
            gemm_pair(psd, vh, xT, at, rhT)
            ot = outp.tile([128, B], F32, tag="out")
            for nb in range(NB):
                zt = actp.tile([128, NBS], BF16, tag="act")
                nc.scalar.activation(
                    zt[:],
                    psz[nb][:],
                    AFT.Sigmoid,
                    bias=bias_sb[:, jt : jt + 1],
                    scale=INV_SCALE,
                )
                dt_ = actp.tile([128, NBS], BF16, tag="act")
                nc.scalar.activation(
                    dt_[:], psd[nb][:], AFT.Tanh, bias=bias_sb[:, 16 + jt : 17 + jt]
                )
                zdh = tmpp.tile([128, NBS], F32, tag="zdh")
                nc.vector.tensor_mul(zdh[:], zt[:], dt_[:])
                # out = (z*dh) * eps + h
                nc.vector.scalar_tensor_tensor(
                    ot[:, ds(nb * NBS, NBS)],
                    zdh[:],
                    float(eps),
                    hTb[:, jt, ds(nb * NBS, NBS)],
                    op0=ALU.mult,
                    op1=ALU.add,
                )
                nc.sync.dma_start(
                    out=out_d[ts(jt, 128), ds(nb * NBS, NBS)],
                    in_=ot[:, ds(nb * NBS, NBS)],
                )

    _dedupe_ldweights(nc)
    nc.compile()
    return nc


def _dedupe_ldweights(nc):
    """Drop back-to-back InstLdweights with identical weight APs.

    Tile legalization splits every bf16 matmul into LDWEIGHTS+MATMUL even when
    consecutive matmuls share the stationary operand. The PE executes its
    stream in order, so a repeated load of the same weights is pure overhead
    (~128 cycles per 512-cycle matmul). Only drops loads that carry no
    semaphore waits/updates; the explicit pe-order dep chain built in _build
    guarantees groups sharing weights are contiguous in the stream.
    """
    removed = 0
    for blk in nc.m.functions[0].blocks:
        new = []
        last_key = None
        for i in blk.instructions:
            if i.engine == mybir.EngineType.PE:
                if isinstance(i, mybir.InstLdweights):
                    si = i.sync_info
                    clean = si is None or (not si.on_wait and not si.on_update)
                    key = str(i.ins[0])
                    if clean and key == last_key:
                        removed += 1
                        continue
                    last_key = key
                elif not isinstance(i, mybir.InstMatmult):
                    last_key = None
            new.append(i)
        blk.instructions[:] = new
    return removed


def _get_nc(eps: float):
    key = float(eps)
    if key not in _nc_cache:
        _nc_cache[key] = _build(key)
    return _nc_cache[key]


def _block_weight(wT, dtype, scale=1.0):
    # [1024, 1024] (contraction-major) -> [jt, p, c, j] st. blk[jt,p,c,j] = wT[c*128+p, jt*128+j]
    blk = wT.reshape(KC, 128, JT, 128).transpose(2, 1, 0, 3)
    if scale != 1.0:
        blk = blk * scale
    return np.ascontiguousarray(blk).astype(dtype)


def _block_data(m):
    # per-core [B, 1024] -> [p, c, b] st. blk[p,c,b] = m[b, c*128+p]
    return np.ascontiguousarray(m.T.reshape(KC, 128, B).transpose(1, 0, 2))


def _prep_in_maps(x, h_prev, W_z, b_z, U_z, W_r, b_r, U_r, V_h, b_h, W_h):
    BF = ml_dtypes.bfloat16
    F8 = ml_dtypes.float8_e4m3
    x16 = np.asarray(x, np.float32).astype(BF)
    h16 = np.asarray(h_prev, np.float32).astype(BF)
    x8 = (np.asarray(x, np.float32) * SCALE_X).astype(F8)
    h8 = (np.asarray(h_prev, np.float32) * SCALE_X).astype(F8)

    A = W_h - W_h.T - GAMMA * np.eye(H, dtype=np.float32)
    shared = {
        "wz8": _block_weight(W_z.T, F8, SCALE_W),
        "uz8": _block_weight(U_z.T, F8, SCALE_W),
        "wr8": _block_weight(W_r.T, F8, SCALE_W),
        "ur8": _block_weight(U_r.T, F8, SCALE_W),
        "vhT": _block_weight(V_h.T, BF),
        "aT": _block_weight(A.T, BF),
        "biases": np.ascontiguousarray(
            np.concatenate(
                [
                    b_z.reshape(JT, 128).T,
                    b_r.reshape(JT, 128).T,
                    b_h.reshape(JT, 128).T,
                ],
                axis=1,
            ).astype(np.float32)
        ),
    }
    in_maps = []
    for c in range(N_CORES):
        sl = slice(c * B, (c + 1) * B)
        in_maps.append(
            {
                "xT": _block_data(x16[sl]),
                "hT": _block_data(h16[sl]),
                "xT8": _block_data(x8[sl]),
                "hT8": _block_data(h8[sl]),
                **shared,
            }
        )
    return in_maps


def run(inputs, trace=False):
    """Returns (full_output [16384,1024] f32, BassKernelResults)."""
    np_in = {k: np.asarray(v, np.float32) for k, v in inputs.items()}
    eps = float(np_in.pop("epsilon"))
    in_maps = _prep_in_maps(**np_in)
    nc = _get_nc(eps)
    res = run_bass_kernel_spmd(
        nc, in_maps, core_ids=list(range(N_CORES)), trace=trace
    )
    out = np.empty((BATCH, H), np.float32)
    for c in range(N_CORES):
        out[c * B : (c + 1) * B, :] = res.results[c]["out"].T
    return out, res


def kernel(**inputs) -> np.ndarray:
    out, _ = run(inputs, trace=False)
    return out


# revision 22
# speedup vs baseline: 1.6838x; 1.0433x over previous
"""AGRU cell (antisymmetric GRU) forward on 8 TRN2 NeuronCores.

Data-parallel: batch 16384 is sharded 2048 rows/core; the six 1024x1024
weight matrices are replicated. No cross-core communication.

Everything on-device is computed in "hidden-major" (transposed) layout:
    zT = sigmoid(Wz @ xT + Uz @ hT + bz)        [H, B]
    rT = sigmoid(Wr @ xT + Ur @ hT + br)
    rhT = rT * hT
    dhT = tanh(Vh @ xT + A @ rhT + bh)
    outT = hT + eps * zT * dhT
so every matmul has the (pre-transposed, host-prepared) weight tile as the
stationary operand and xT/hT/rhT as the moving operand, and nothing ever
needs an on-device transpose.  The host transposes each core's [1024, 2048]
result back when assembling the full output.

Matmuls run in bf16 (1 cycle/row on TRN2 vs 4 for fp32) with fp32 PSUM
accumulation; the final residual add is done in fp32.
"""

import sys

sys.path.insert(0, "/opt/trn_rl_repo")

import numpy as np
import ml_dtypes

from contextlib import ExitStack

import concourse.bass as bass
import concourse.mybir as mybir
from concourse import bacc, tile
from concourse.bass import ds, ts
from concourse.bass_utils import run_bass_kernel_spmd
from concourse.tile_rust import add_dep_helper

BF16 = mybir.dt.bfloat16
FP8 = mybir.dt.float8e4
F32 = mybir.dt.float32
AFT = mybir.ActivationFunctionType
ALU = mybir.AluOpType
DR = mybir.MatmulPerfMode.DoubleRow

# fp8 pre-scaling for the sigmoid-gate GEMMs (z, r): data*16, weights*256,
# compensated by activation scale 1/(16*256).
SCALE_X = 16.0
SCALE_W = 256.0
INV_SCALE = 1.0 / (SCALE_X * SCALE_W)

N_CORES = 8
BATCH = 16384
B = BATCH // N_CORES  # per-core batch shard (2048)
H = 1024  # hidden == input size
KC = H // 128  # contraction chunks (8)
JT = H // 128  # output row tiles (8)
NB = 4  # moving-dim (batch) blocks per psum bank
NBS = B // NB  # 512 columns per matmul
GAMMA = 0.01

_nc_cache = {}


def _build(eps: float):
    """Build + compile the single-core Tile program (same graph on all cores)."""
    nc = bacc.Bacc("TRN2", target_bir_lowering=False, debug=False)

    xT_d = nc.dram_tensor("xT", [128, KC, B], BF16, kind="ExternalInput")
    hT_d = nc.dram_tensor("hT", [128, KC, B], BF16, kind="ExternalInput")
    xT8_d = nc.dram_tensor("xT8", [128, KC, B], FP8, kind="ExternalInput")
    hT8_d = nc.dram_tensor("hT8", [128, KC, B], FP8, kind="ExternalInput")
    w_d = {
        name: nc.dram_tensor(name, [JT, 128, KC, 128], BF16, kind="ExternalInput")
        for name in ["vhT", "aT"]
    }
    w8_d = {
        name: nc.dram_tensor(name, [JT, 128, KC, 128], FP8, kind="ExternalInput")
        for name in ["wz8", "uz8", "wr8", "ur8"]
    }
    bias_d = nc.dram_tensor("biases", [128, 24], F32, kind="ExternalInput")
    out_d = nc.dram_tensor("out", [H, B], F32, kind="ExternalOutput")

    with tile.TileContext(nc) as tc, ExitStack() as ctx:
        singles = ctx.enter_context(tc.tile_pool(name="singles", bufs=1))
        wpool = ctx.enter_context(tc.tile_pool(name="wpool", bufs=8))
        psum = ctx.enter_context(tc.tile_pool(name="psum", bufs=8, space="PSUM"))
        actp = ctx.enter_context(tc.tile_pool(name="actp", bufs=10))
        tmpp = ctx.enter_context(tc.tile_pool(name="tmpp", bufs=4))
        outp = ctx.enter_context(tc.tile_pool(name="outp", bufs=2))

        xT = singles.tile([128, KC, B], BF16)
        hTb = singles.tile([128, KC, B], BF16)
        xT8 = singles.tile([128, KC, B], FP8)
        hT8 = singles.tile([128, KC, B], FP8)
        rhT = singles.tile([128, KC, B], BF16)
        bias_sb = singles.tile([128, 24], F32)

        def load_w(name, jt):
            # gpsimd (SWDGE): keeps weight blocks off the Sync HWDGE queue,
            # which is busy issuing the bulk x/h stream.
            fp8 = name in w8_d
            w = wpool.tile([128, KC, 128], FP8 if fp8 else BF16, tag="w")
            nc.gpsimd.dma_start(out=w[:], in_=(w8_d[name] if fp8 else w_d[name])[jt])
            return w

        # Weights for the first row-block go out first so the PE isn't stuck
        # behind the x/h stream on the DMA queues.
        wr0 = load_w("wr8", 0)
        ur0 = load_w("ur8", 0)
        for c in range(KC):
            nc.sync.dma_start(out=xT8[:, c, :], in_=xT8_d[:, c, :])
            nc.sync.dma_start(out=hT8[:, c, :], in_=hT8_d[:, c, :])
        nc.sync.dma_start(out=bias_sb[:], in_=bias_d[:])
        # bf16 h before bf16 x: phase-1's rh-multiplies consume hTb chunks
        # almost immediately, while bf16 x is only needed in phase 2.
        for c in range(KC):
            nc.sync.dma_start(out=hTb[:, c, :], in_=hT_d[:, c, :])
        for c in range(KC):
            nc.sync.dma_start(out=xT[:, c, :], in_=xT_d[:, c, :])

        # All PE matmuls are chained in program order (ordering-only deps) so
        # that groups of matmuls sharing a stationary operand stay contiguous:
        # followers in each group skip their LDWEIGHTS (ldweights=False) and
        # reuse the weights already in the array.
        prev_mm = [None]

        def mm(psum_ap, w_ap, rhs_ap, start, stop, reload_w, perf_mode=None):
            bi = nc.tensor.matmul(
                psum_ap, w_ap, rhs_ap, start=start, stop=stop, perf_mode=perf_mode
            )
            if not reload_w:
                bi.ins.ldweights = False
            if prev_mm[0] is not None:
                add_dep_helper(bi.ins, prev_mm[0], False, "pe-order")
            prev_mm[0] = bi.ins
            return bi

        def gemm_pair(psums, wA, rhsA, wB, rhsB):
            # psums[nb] += wA[:,k,:].T @ rhsA[:,k,nb] summed over k, then wB/rhsB
            for k in range(KC):
                for nb in range(NB):
                    mm(
                        psums[nb][:],
                        wA[:, k, :],
                        rhsA[:, k, ds(nb * NBS, NBS)],
                        start=(k == 0),
                        stop=False,
                        reload_w=(nb == 0),
                    )
            for k in range(KC):
                for nb in range(NB):
                    mm(
                        psums[nb][:],
                        wB[:, k, :],
                        rhsB[:, k, ds(nb * NBS, NBS)],
                        start=False,
                        stop=(k == KC - 1),
                        reload_w=(nb == 0),
                    )

        def gemm_pair_fp8(psums, wA, rhsA, wB, rhsB):
            # fp8 DoubleRow: each matmul covers two 128-row contraction chunks
            for k in range(0, KC, 2):
                for nb in range(NB):
                    mm(
                        psums[nb][:],
                        wA[:, k : k + 2, :],
                        rhsA[:, k : k + 2, ds(nb * NBS, NBS)],
                        start=(k == 0),
                        stop=False,
                        reload_w=(nb == 0),
                        perf_mode=DR,
                    )
            for k in range(0, KC, 2):
                for nb in range(NB):
                    mm(
                        psums[nb][:],
                        wB[:, k : k + 2, :],
                        rhsB[:, k : k + 2, ds(nb * NBS, NBS)],
                        start=False,
                        stop=(k == KC - 2),
                        reload_w=(nb == 0),
                        perf_mode=DR,
                    )

        # ---- phase 1: r gate (hidden-major, fp8), rhT = sigmoid(...) * hT ----
        for jt in range(JT):
            if jt == 0:
                wr, ur = wr0, ur0
            else:
                wr = load_w("wr8", jt)
                ur = load_w("ur8", jt)
            ps = [
                psum.tile([128, NBS], F32, tag="ps", name=f"ps_r{jt}_{i}")
                for i in range(NB)
            ]
            gemm_pair_fp8(ps, wr, xT8, ur, hT8)
            for nb in range(NB):
                rt = actp.tile([128, NBS], BF16, tag="act")
                nc.scalar.activation(
                    rt[:],
                    ps[nb][:],
                    AFT.Sigmoid,
                    bias=bias_sb[:, 8 + jt : 9 + jt],
                    scale=INV_SCALE,
                )
                nc.vector.tensor_mul(
                    rhT[:, jt, ds(nb * NBS, NBS)],
                    rt[:],
                    hTb[:, jt, ds(nb * NBS, NBS)],
                )

        # ---- phase 2: z gate (fp8) + delta_h (bf16) + residual, per jt ----
        for jt in range(JT):
            wz = load_w("wz8", jt)
            uz = load_w("uz8", jt)
            vh = load_w("vhT", jt)
            at = load_w("aT", jt)
            psz = [
                psum.tile([128, NBS], F32, tag="ps", name=f"ps_z{jt}_{i}")
                for i in range(NB)
            ]
            gemm_pair_fp8(psz, wz, xT8, uz, hT8)
            psd = [
                psum.tile([128, NBS], F32, tag="ps", name=f"ps_d{jt}_{i}")
                for i in range(NB)
            ]
            gemm_pair(psd, vh, xT, at, rhT)
            ot = outp.tile([128, B], F32, tag="out")
            for nb in range(NB):
                zt = actp.tile([128, NBS], BF16, tag="act")
                nc.scalar.activation(
                    zt[:],
                    psz[nb][:],
                    AFT.Sigmoid,
                    bias=bias_sb[:, jt : jt + 1],
                    scale=INV_SCALE,
                )
                dt_ = actp.tile([128, NBS], BF16, tag="act")
                nc.scalar.activation(
                    dt_[:], psd[nb][:], AFT.Tanh, bias=bias_sb[:, 16 + jt : 17 + jt]
                )
                zdh = tmpp.tile([128, NBS], F32, tag="zdh")
                nc.vector.tensor_mul(zdh[:], zt[:], dt_[:])
                # out = (z*dh) * eps + h
                nc.vector.scalar_tensor_tensor(
                    ot[:, ds(nb * NBS, NBS)],
                    zdh[:],
                    float(eps),
                    hTb[:, jt, ds(nb * NBS, NBS)],
                    op0=ALU.mult,
                    op1=ALU.add,
                )
                nc.sync.dma_start(
                    out=out_d[ts(jt, 128), ds(nb * NBS, NBS)],
                    in_=ot[:, ds(nb * NBS, NBS)],
                )

    _dedupe_ldweights(nc)
    nc.compile()
    return nc


def _dedupe_ldweights(nc):
    """Drop back-to-back InstLdweights with identical weight APs.

    Tile legalization splits every bf16 matmul into LDWEIGHTS+MATMUL even when
    consecutive matmuls share the stationary operand. The PE executes its
    stream in order, so a repeated load of the same weights is pure overhead
    (~128 cycles per 512-cycle matmul). Only drops loads that carry no
    semaphore waits/updates; the explicit pe-order dep chain built in _build
    guarantees groups sharing weights are contiguous in the stream.
    """
    removed = 0
    for blk in nc.m.functions[0].blocks:
        new = []
        last_key = None
        for i in blk.instructions:
            if i.engine == mybir.EngineType.PE:
                if isinstance(i, mybir.InstLdweights):
                    si = i.sync_info
                    clean = si is None or (not si.on_wait and not si.on_update)
                    key = str(i.ins[0])
                    if clean and key == last_key:
                        removed += 1
                        continue
                    last_key = key
                elif not isinstance(i, mybir.InstMatmult):
                    last_key = None
            new.append(i)
        blk.instructions[:] = new
    return removed


def _get_nc(eps: float):
    key = float(eps)
    if key not in _nc_cache:
        _nc_cache[key] = _build(key)
    return _nc_cache[key]


def _block_weight(wT, dtype, scale=1.0):
    # [1024, 1024] (contraction-major) -> [jt, p, c, j] st. blk[jt,p,c,j] = wT[c*128+p, jt*128+j]
    blk = wT.reshape(KC, 128, JT, 128).transpose(2, 1, 0, 3)
    if scale != 1.0:
        blk = blk * scale
    return np.ascontiguousarray(blk).astype(dtype)


def _block_data(m):
    # per-core [B, 1024] -> [p, c, b] st. blk[p,c,b] = m[b, c*128+p]
    return np.ascontiguousarray(m.T.reshape(KC, 128, B).transpose(1, 0, 2))


def _prep_in_maps(x, h_prev, W_z, b_z, U_z, W_r, b_r, U_r, V_h, b_h, W_h):
    BF = ml_dtypes.bfloat16
    F8 = ml_dtypes.float8_e4m3
    x16 = np.asarray(x, np.float32).astype(BF)
    h16 = np.asarray(h_prev, np.float32).astype(BF)
    x8 = (np.asarray(x, np.float32) * SCALE_X).astype(F8)
    h8 = (np.asarray(h_prev, np.float32) * SCALE_X).astype(F8)

    A = W_h - W_h.T - GAMMA * np.eye(H, dtype=np.float32)
    shared = {
        "wz8": _block_weight(W_z.T, F8, SCALE_W),
        "uz8": _block_weight(U_z.T, F8, SCALE_W),
        "wr8": _block_weight(W_r.T, F8, SCALE_W),
        "ur8": _block_weight(U_r.T, F8, SCALE_W),
        "vhT": _block_weight(V_h.T, BF),
        "aT": _block_weight(A.T, BF),
        "biases": np.ascontiguousarray(
            np.concatenate(
                [
                    b_z.reshape(JT, 128).T,
                    b_r.reshape(JT, 128).T,
                    b_h.reshape(JT, 128).T,
                ],
                axis=1,
            ).astype(np.float32)
        ),
    }
    in_maps = []
    for c in range(N_CORES):
        sl = slice(c * B, (c + 1) * B)
        in_maps.append(
            {
                "xT": _block_data(x16[sl]),
                "hT": _block_data(h16[sl]),
                "xT8": _block_data(x8[sl]),
                "hT8": _block_data(h8[sl]),
                **shared,
            }
        )
    return in_maps


def run(inputs, trace=False):
    """Returns (full_output [16384,1024] f32, BassKernelResults)."""
    np_in = {k: np.asarray(v, np.float32) for k, v in inputs.items()}
    eps = float(np_in.pop("epsilon"))
    in_maps = _prep_in_maps(**np_in)
    nc = _get_nc(eps)
    res = run_bass_kernel_spmd(
        nc, in_maps, core_ids=list(range(N_CORES)), trace=trace
    )
    out = np.empty((BATCH, H), np.float32)
    for c in range(N_CORES):
        out[c * B : (c + 1) * B, :] = res.results[c]["out"].T
    return out, res


def kernel(**inputs) -> np.ndarray:
    out, _ = run(inputs, trace=False)
    return out


# revision 29
# speedup vs baseline: 1.8323x; 1.0882x over previous
"""AGRU cell (antisymmetric GRU) forward on 8 TRN2 NeuronCores.

Data-parallel: batch 16384 is sharded 2048 rows/core; the six 1024x1024
weight matrices are replicated. No cross-core communication.

Everything on-device is computed in "hidden-major" (transposed) layout:
    zT = sigmoid(Wz @ xT + Uz @ hT + bz)        [H, B]
    rT = sigmoid(Wr @ xT + Ur @ hT + br)
    rhT = rT * hT
    dhT = tanh(Vh @ xT + A @ rhT + bh)
    outT = hT + eps * zT * dhT
so every matmul has the (pre-transposed, host-prepared) weight tile as the
stationary operand and xT/hT/rhT as the moving operand, and nothing ever
needs an on-device transpose.  The host transposes each core's [1024, 2048]
result back when assembling the full output.

Matmuls run in bf16 (1 cycle/row on TRN2 vs 4 for fp32) with fp32 PSUM
accumulation; the final residual add is done in fp32.
"""

import sys

sys.path.insert(0, "/opt/trn_rl_repo")

import numpy as np
import ml_dtypes

from contextlib import ExitStack

import concourse.bass as bass
import concourse.mybir as mybir
from concourse import bacc, tile
from concourse.bass import ds, ts
from concourse.bass_utils import run_bass_kernel_spmd
from concourse.tile_rust import add_dep_helper

BF16 = mybir.dt.bfloat16
FP8 = mybir.dt.float8e4
F32 = mybir.dt.float32
AFT = mybir.ActivationFunctionType
ALU = mybir.AluOpType
DR = mybir.MatmulPerfMode.DoubleRow

# fp8 pre-scaling for the sigmoid-gate GEMMs (z, r): data*16, weights*256,
# compensated by activation scale 1/(16*256).
SCALE_X = 16.0
SCALE_W = 256.0
INV_SCALE = 1.0 / (SCALE_X * SCALE_W)

N_CORES = 8
BATCH = 16384
B = BATCH // N_CORES  # per-core batch shard (2048)
H = 1024  # hidden == input size
KC = H // 128  # contraction chunks (8)
JT = H // 128  # output row tiles (8)
NB = 4  # moving-dim (batch) blocks per psum bank
NBS = B // NB  # 512 columns per matmul
GAMMA = 0.01

_nc_cache = {}


def _build(eps: float):
    """Build + compile the single-core Tile program (same graph on all cores)."""
    nc = bacc.Bacc("TRN2", target_bir_lowering=False, debug=False)

    xT_d = nc.dram_tensor("xT", [128, KC, B], BF16, kind="ExternalInput")
    hT_d = nc.dram_tensor("hT", [128, KC, B], BF16, kind="ExternalInput")
    xT8_d = nc.dram_tensor("xT8", [128, KC, B], FP8, kind="ExternalInput")
    hT8_d = nc.dram_tensor("hT8", [128, KC, B], FP8, kind="ExternalInput")
    w_d = {
        name: nc.dram_tensor(name, [JT, 128, KC, 128], BF16, kind="ExternalInput")
        for name in ["vhT"]
    }
    w8_d = {
        name: nc.dram_tensor(name, [JT, 128, KC, 128], FP8, kind="ExternalInput")
        for name in ["wz8", "uz8", "wr8", "ur8", "at8"]
    }
    bias_d = nc.dram_tensor("biases", [128, 24], F32, kind="ExternalInput")
    out_d = nc.dram_tensor("out", [H, B], F32, kind="ExternalOutput")

    with tile.TileContext(nc) as tc, ExitStack() as ctx:
        singles = ctx.enter_context(tc.tile_pool(name="singles", bufs=1))
        wpool = ctx.enter_context(tc.tile_pool(name="wpool", bufs=8))
        psum = ctx.enter_context(tc.tile_pool(name="psum", bufs=8, space="PSUM"))
        actp = ctx.enter_context(tc.tile_pool(name="actp", bufs=10))
        tmpp = ctx.enter_context(tc.tile_pool(name="tmpp", bufs=5))
        outp = ctx.enter_context(tc.tile_pool(name="outp", bufs=2))

        xT = singles.tile([128, KC, B], BF16)
        hTb = singles.tile([128, KC, B], BF16)
        xT8 = singles.tile([128, KC, B], FP8)
        hT8 = singles.tile([128, KC, B], FP8)
        rhT8 = singles.tile([128, KC, B], FP8)
        bias_sb = singles.tile([128, 24], F32)

        def load_w(name, jt):
            # gpsimd (SWDGE): keeps weight blocks off the Sync HWDGE queue,
            # which is busy issuing the bulk x/h stream.
            fp8 = name in w8_d
            w = wpool.tile([128, KC, 128], FP8 if fp8 else BF16, tag="w")
            nc.gpsimd.dma_start(out=w[:], in_=(w8_d[name] if fp8 else w_d[name])[jt])
            return w

        # Weights for the first row-block go out first so the PE isn't stuck
        # behind the x/h stream on the DMA queues.
        wr0 = load_w("wr8", 0)
        ur0 = load_w("ur8", 0)
        for c in range(KC):
            nc.sync.dma_start(out=xT8[:, c, :], in_=xT8_d[:, c, :])
            nc.sync.dma_start(out=hT8[:, c, :], in_=hT8_d[:, c, :])
        nc.sync.dma_start(out=bias_sb[:], in_=bias_d[:])
        # bf16 h before bf16 x: phase-1's rh-multiplies consume hTb chunks
        # almost immediately, while bf16 x is only needed in phase 2.
        for c in range(KC):
            nc.sync.dma_start(out=hTb[:, c, :], in_=hT_d[:, c, :])
        for c in range(KC):
            nc.sync.dma_start(out=xT[:, c, :], in_=xT_d[:, c, :])

        # All PE matmuls are chained in program order (ordering-only deps) so
        # that groups of matmuls sharing a stationary operand stay contiguous:
        # followers in each group skip their LDWEIGHTS (ldweights=False) and
        # reuse the weights already in the array.
        prev_mm = [None]

        def mm(psum_ap, w_ap, rhs_ap, start, stop, reload_w, perf_mode=None):
            bi = nc.tensor.matmul(
                psum_ap, w_ap, rhs_ap, start=start, stop=stop, perf_mode=perf_mode
            )
            if not reload_w:
                bi.ins.ldweights = False
            if prev_mm[0] is not None:
                add_dep_helper(bi.ins, prev_mm[0], False, "pe-order")
            prev_mm[0] = bi.ins
            return bi

        def gemm_pair(psums, wA, rhsA, wB, rhsB):
            # psums[nb] += wA[:,k,:].T @ rhsA[:,k,nb] summed over k, then wB/rhsB
            for k in range(KC):
                for nb in range(NB):
                    mm(
                        psums[nb][:],
                        wA[:, k, :],
                        rhsA[:, k, ds(nb * NBS, NBS)],
                        start=(k == 0),
                        stop=False,
                        reload_w=(nb == 0),
                    )
            for k in range(KC):
                for nb in range(NB):
                    mm(
                        psums[nb][:],
                        wB[:, k, :],
                        rhsB[:, k, ds(nb * NBS, NBS)],
                        start=False,
                        stop=(k == KC - 1),
                        reload_w=(nb == 0),
                    )

        def gemm_pair_fp8(psums, wA, rhsA, wB, rhsB):
            # fp8 DoubleRow: each matmul covers two 128-row contraction chunks
            for k in range(0, KC, 2):
                for nb in range(NB):
                    mm(
                        psums[nb][:],
                        wA[:, k : k + 2, :],
                        rhsA[:, k : k + 2, ds(nb * NBS, NBS)],
                        start=(k == 0),
                        stop=False,
                        reload_w=(nb == 0),
                        perf_mode=DR,
                    )
            for k in range(0, KC, 2):
                for nb in range(NB):
                    mm(
                        psums[nb][:],
                        wB[:, k : k + 2, :],
                        rhsB[:, k : k + 2, ds(nb * NBS, NBS)],
                        start=False,
                        stop=(k == KC - 2),
                        reload_w=(nb == 0),
                        perf_mode=DR,
                    )

        # ---- phase 1: r gate (hidden-major, fp8), rhT = sigmoid(...) * hT ----
        for jt in range(JT):
            if jt == 0:
                wr, ur = wr0, ur0
            else:
                wr = load_w("wr8", jt)
                ur = load_w("ur8", jt)
            ps = [
                psum.tile([128, NBS], F32, tag="ps", name=f"ps_r{jt}_{i}")
                for i in range(NB)
            ]
            gemm_pair_fp8(ps, wr, xT8, ur, hT8)
            for nb in range(NB):
                rt = actp.tile([128, NBS], BF16, tag="act")
                nc.scalar.activation(
                    rt[:],
                    ps[nb][:],
                    AFT.Sigmoid,
                    bias=bias_sb[:, 8 + jt : 9 + jt],
                    scale=INV_SCALE,
                )
                # rh in scaled fp8 for the DoubleRow A-matmul: (r*16)*h
                nc.vector.scalar_tensor_tensor(
                    rhT8[:, jt, ds(nb * NBS, NBS)],
                    rt[:],
                    SCALE_X,
                    hTb[:, jt, ds(nb * NBS, NBS)],
                    op0=ALU.mult,
                    op1=ALU.mult,
                )

        # ---- phase 2: z gate (fp8) + delta_h (bf16) + residual, per jt ----
        for jt in range(JT):
            wz = load_w("wz8", jt)
            uz = load_w("uz8", jt)
            vh = load_w("vhT", jt)
            at = load_w("at8", jt)
            psz = [
                psum.tile([128, NBS], F32, tag="ps", name=f"ps_z{jt}_{i}")
                for i in range(NB)
            ]
            gemm_pair_fp8(psz, wz, xT8, uz, hT8)
            # dh-pre: V_h@x in bf16 (V_h host-scaled by 4096) and A@(r*h) in
            # fp8 DoubleRow (scales 16*256) — separate PSUM groups (mixing
            # perf modes in one accumulation group faults the device), summed
            # on the DVE.
            psv = [
                psum.tile([128, NBS], F32, tag="ps", name=f"ps_v{jt}_{i}")
                for i in range(NB)
            ]
            for k in range(KC):
                for nb in range(NB):
                    mm(
                        psv[nb][:],
                        vh[:, k, :],
                        xT[:, k, ds(nb * NBS, NBS)],
                        start=(k == 0),
                        stop=(k == KC - 1),
                        reload_w=(nb == 0),
                    )
            vhs = []
            for nb in range(NB):
                t = tmpp.tile([128, NBS], F32, tag="vhs", name=f"vhs{jt}_{nb}")
                nc.vector.tensor_copy(t[:], psv[nb][:])
                vhs.append(t)
            psd = [
                psum.tile([128, NBS], F32, tag="ps", name=f"ps_d{jt}_{i}")
                for i in range(NB)
            ]
            for k in range(0, KC, 2):
                for nb in range(NB):
                    mm(
                        psd[nb][:],
                        at[:, k : k + 2, :],
                        rhT8[:, k : k + 2, ds(nb * NBS, NBS)],
                        start=(k == 0),
                        stop=(k == KC - 2),
                        reload_w=(nb == 0),
                        perf_mode=DR,
                    )
            ot = outp.tile([128, B], F32, tag="out")
            for nb in range(NB):
                zt = actp.tile([128, NBS], BF16, tag="act")
                nc.scalar.activation(
                    zt[:],
                    psz[nb][:],
                    AFT.Sigmoid,
                    bias=bias_sb[:, jt : jt + 1],
                    scale=INV_SCALE,
                )
                dsum = tmpp.tile([128, NBS], F32, tag="dsum")
                nc.vector.tensor_add(dsum[:], psd[nb][:], vhs[nb][:])
                dt_ = actp.tile([128, NBS], BF16, tag="act")
                nc.scalar.activation(
                    dt_[:],
                    dsum[:],
                    AFT.Tanh,
                    bias=bias_sb[:, 16 + jt : 17 + jt],
                    scale=INV_SCALE,
                )
                zdh = tmpp.tile([128, NBS], F32, tag="zdh")
                nc.vector.tensor_mul(zdh[:], zt[:], dt_[:])
                # out = (z*dh) * eps + h
                nc.vector.scalar_tensor_tensor(
                    ot[:, ds(nb * NBS, NBS)],
                    zdh[:],
                    float(eps),
                    hTb[:, jt, ds(nb * NBS, NBS)],
                    op0=ALU.mult,
                    op1=ALU.add,
                )
                nc.sync.dma_start(
                    out=out_d[ts(jt, 128), ds(nb * NBS, NBS)],
                    in_=ot[:, ds(nb * NBS, NBS)],
                )

    _dedupe_ldweights(nc)
    nc.compile()
    return nc


def _dedupe_ldweights(nc):
    """Drop back-to-back InstLdweights with identical weight APs.

    Tile legalization splits every bf16 matmul into LDWEIGHTS+MATMUL even when
    consecutive matmuls share the stationary operand. The PE executes its
    stream in order, so a repeated load of the same weights is pure overhead
    (~128 cycles per 512-cycle matmul). Only drops loads that carry no
    semaphore waits/updates; the explicit pe-order dep chain built in _build
    guarantees groups sharing weights are contiguous in the stream.
    """
    removed = 0
    for blk in nc.m.functions[0].blocks:
        new = []
        last_key = None
        for i in blk.instructions:
            if i.engine == mybir.EngineType.PE:
                if isinstance(i, mybir.InstLdweights):
                    si = i.sync_info
                    clean = si is None or (not si.on_wait and not si.on_update)
                    key = str(i.ins[0])
                    if clean and key == last_key:
                        removed += 1
                        continue
                    last_key = key
                elif not isinstance(i, mybir.InstMatmult):
                    last_key = None
            new.append(i)
        blk.instructions[:] = new
    return removed


def _get_nc(eps: float):
    key = float(eps)
    if key not in _nc_cache:
        _nc_cache[key] = _build(key)
    return _nc_cache[key]


def _block_weight(wT, dtype, scale=1.0):
    # [1024, 1024] (contraction-major) -> [jt, p, c, j] st. blk[jt,p,c,j] = wT[c*128+p, jt*128+j]
    blk = wT.reshape(KC, 128, JT, 128).transpose(2, 1, 0, 3)
    if scale != 1.0:
        blk = blk * scale
    return np.ascontiguousarray(blk).astype(dtype)


def _block_data(m):
    # per-core [B, 1024] -> [p, c, b] st. blk[p,c,b] = m[b, c*128+p]
    return np.ascontiguousarray(m.T.reshape(KC, 128, B).transpose(1, 0, 2))


def _prep_in_maps(x, h_prev, W_z, b_z, U_z, W_r, b_r, U_r, V_h, b_h, W_h):
    BF = ml_dtypes.bfloat16
    F8 = ml_dtypes.float8_e4m3
    x16 = np.asarray(x, np.float32).astype(BF)
    h16 = np.asarray(h_prev, np.float32).astype(BF)
    x8 = (np.asarray(x, np.float32) * SCALE_X).astype(F8)
    h8 = (np.asarray(h_prev, np.float32) * SCALE_X).astype(F8)

    A = W_h - W_h.T - GAMMA * np.eye(H, dtype=np.float32)
    shared = {
        "wz8": _block_weight(W_z.T, F8, SCALE_W),
        "uz8": _block_weight(U_z.T, F8, SCALE_W),
        "wr8": _block_weight(W_r.T, F8, SCALE_W),
        "ur8": _block_weight(U_r.T, F8, SCALE_W),
        "at8": _block_weight(A.T, F8, SCALE_W),
        "vhT": _block_weight(V_h.T, BF, SCALE_X * SCALE_W),
        "biases": np.ascontiguousarray(
            np.concatenate(
                [
                    b_z.reshape(JT, 128).T,
                    b_r.reshape(JT, 128).T,
                    b_h.reshape(JT, 128).T,
                ],
                axis=1,
            ).astype(np.float32)
        ),
    }
    in_maps = []
    for c in range(N_CORES):
        sl = slice(c * B, (c + 1) * B)
        in_maps.append(
            {
                "xT": _block_data(x16[sl]),
                "hT": _block_data(h16[sl]),
                "xT8": _block_data(x8[sl]),
                "hT8": _block_data(h8[sl]),
                **shared,
            }
        )
    return in_maps


def run(inputs, trace=False):
    """Returns (full_output [16384,1024] f32, BassKernelResults)."""
    np_in = {k: np.asarray(v, np.float32) for k, v in inputs.items()}
    eps = float(np_in.pop("epsilon"))
    in_maps = _prep_in_maps(**np_in)
    nc = _get_nc(eps)
    res = run_bass_kernel_spmd(
        nc, in_maps, core_ids=list(range(N_CORES)), trace=trace
    )
    out = np.empty((BATCH, H), np.float32)
    for c in range(N_CORES):
        out[c * B : (c + 1) * B, :] = res.results[c]["out"].T
    return out, res


def kernel(**inputs) -> np.ndarray:
    out, _ = run(inputs, trace=False)
    return out


# revision 34
# speedup vs baseline: 1.8527x; 1.0111x over previous
"""AGRU cell (antisymmetric GRU) forward on 8 TRN2 NeuronCores.

Data-parallel: batch 16384 is sharded 2048 rows/core; the six 1024x1024
weight matrices are replicated. No cross-core communication.

Everything on-device is computed in "hidden-major" (transposed) layout:
    zT = sigmoid(Wz @ xT + Uz @ hT + bz)        [H, B]
    rT = sigmoid(Wr @ xT + Ur @ hT + br)
    rhT = rT * hT
    dhT = tanh(Vh @ xT + A @ rhT + bh)
    outT = hT + eps * zT * dhT
so every matmul has the (pre-transposed, host-prepared) weight tile as the
stationary operand and xT/hT/rhT as the moving operand, and nothing ever
needs an on-device transpose.  The host transposes each core's [1024, 2048]
result back when assembling the full output.

Matmuls run in bf16 (1 cycle/row on TRN2 vs 4 for fp32) with fp32 PSUM
accumulation; the final residual add is done in fp32.
"""

import sys

sys.path.insert(0, "/opt/trn_rl_repo")

import numpy as np
import ml_dtypes

from contextlib import ExitStack

import concourse.bass as bass
import concourse.mybir as mybir
from concourse import bacc, tile
from concourse.bass import ds, ts
from concourse.bass_utils import run_bass_kernel_spmd
from concourse.tile_rust import add_dep_helper

BF16 = mybir.dt.bfloat16
FP8 = mybir.dt.float8e4
F32 = mybir.dt.float32
AFT = mybir.ActivationFunctionType
ALU = mybir.AluOpType
DR = mybir.MatmulPerfMode.DoubleRow

# fp8 pre-scaling for the sigmoid-gate GEMMs (z, r): data*16, weights*256,
# compensated by activation scale 1/(16*256).
SCALE_X = 16.0
SCALE_W = 256.0
INV_SCALE = 1.0 / (SCALE_X * SCALE_W)

N_CORES = 8
BATCH = 16384
B = BATCH // N_CORES  # per-core batch shard (2048)
H = 1024  # hidden == input size
KC = H // 128  # contraction chunks (8)
JT = H // 128  # output row tiles (8)
NB = 4  # moving-dim (batch) blocks per psum bank
NBS = B // NB  # 512 columns per matmul
GAMMA = 0.01

_nc_cache = {}


def _build(eps: float):
    """Build + compile the single-core Tile program (same graph on all cores)."""
    nc = bacc.Bacc("TRN2", target_bir_lowering=False, debug=False)

    xT_d = nc.dram_tensor("xT", [128, KC, B], BF16, kind="ExternalInput")
    hT_d = nc.dram_tensor("hT", [128, KC, B], BF16, kind="ExternalInput")
    # fp8 activations arrive batch-block-major so phase 1 can start on the
    # first 512-column block instead of waiting for the full stream.
    xT8_d = nc.dram_tensor("xT8", [NB, 128, KC, NBS], FP8, kind="ExternalInput")
    hT8_d = nc.dram_tensor("hT8", [NB, 128, KC, NBS], FP8, kind="ExternalInput")
    w_d = {
        name: nc.dram_tensor(name, [JT, 128, KC, 128], BF16, kind="ExternalInput")
        for name in ["vhT"]
    }
    w8_d = {
        name: nc.dram_tensor(name, [JT, 128, KC, 128], FP8, kind="ExternalInput")
        for name in ["wz8", "uz8", "wr8", "ur8", "at8"]
    }
    bias_d = nc.dram_tensor("biases", [128, 24], F32, kind="ExternalInput")
    out_d = nc.dram_tensor("out", [H, B], F32, kind="ExternalOutput")

    with tile.TileContext(nc) as tc, ExitStack() as ctx:
        singles = ctx.enter_context(tc.tile_pool(name="singles", bufs=1))
        wpool = ctx.enter_context(tc.tile_pool(name="wpool", bufs=8))
        psum = ctx.enter_context(tc.tile_pool(name="psum", bufs=8, space="PSUM"))
        actp = ctx.enter_context(tc.tile_pool(name="actp", bufs=10))
        tmpp = ctx.enter_context(tc.tile_pool(name="tmpp", bufs=5))
        outp = ctx.enter_context(tc.tile_pool(name="outp", bufs=2))

        xT = singles.tile([128, KC, B], BF16)
        hTb = singles.tile([128, KC, B], BF16)
        xT8 = singles.tile([128, KC, B], FP8)
        hT8 = singles.tile([128, KC, B], FP8)
        rhT8 = singles.tile([128, KC, B], FP8)
        bias_sb = singles.tile([128, 24], F32)

        def load_w(name, jt):
            # gpsimd (SWDGE): keeps weight blocks off the Sync HWDGE queue,
            # which is busy issuing the bulk x/h stream.
            fp8 = name in w8_d
            w = wpool.tile([128, KC, 128], FP8 if fp8 else BF16, tag="w")
            nc.gpsimd.dma_start(out=w[:], in_=(w8_d[name] if fp8 else w_d[name])[jt])
            return w

        # Weights for the first row-block go out first so the PE isn't stuck
        # behind the x/h stream on the DMA queues.
        wr0 = load_w("wr8", 0)
        ur0 = load_w("ur8", 0)
        for nb in range(NB):
            nc.sync.dma_start(out=xT8[:, :, ds(nb * NBS, NBS)], in_=xT8_d[nb])
            nc.sync.dma_start(out=hT8[:, :, ds(nb * NBS, NBS)], in_=hT8_d[nb])
        nc.sync.dma_start(out=bias_sb[:], in_=bias_d[:])
        # bf16 h before bf16 x: phase-1's rh-multiplies consume hTb chunks
        # almost immediately, while bf16 x is only needed in phase 2.
        for c in range(KC):
            nc.sync.dma_start(out=hTb[:, c, :], in_=hT_d[:, c, :])
        for c in range(KC):
            nc.sync.dma_start(out=xT[:, c, :], in_=xT_d[:, c, :])

        # All PE matmuls are chained in program order (ordering-only deps) so
        # that groups of matmuls sharing a stationary operand stay contiguous:
        # followers in each group skip their LDWEIGHTS (ldweights=False) and
        # reuse the weights already in the array.
        prev_mm = [None]

        def mm(psum_ap, w_ap, rhs_ap, start, stop, reload_w, perf_mode=None):
            bi = nc.tensor.matmul(
                psum_ap, w_ap, rhs_ap, start=start, stop=stop, perf_mode=perf_mode
            )
            if not reload_w:
                bi.ins.ldweights = False
            if prev_mm[0] is not None:
                add_dep_helper(bi.ins, prev_mm[0], False, "pe-order")
            prev_mm[0] = bi.ins
            return bi

        def gemm_pair(psums, wA, rhsA, wB, rhsB):
            # psums[nb] += wA[:,k,:].T @ rhsA[:,k,nb] summed over k, then wB/rhsB
            for k in range(KC):
                for nb in range(NB):
                    mm(
                        psums[nb][:],
                        wA[:, k, :],
                        rhsA[:, k, ds(nb * NBS, NBS)],
                        start=(k == 0),
                        stop=False,
                        reload_w=(nb == 0),
                    )
            for k in range(KC):
                for nb in range(NB):
                    mm(
                        psums[nb][:],
                        wB[:, k, :],
                        rhsB[:, k, ds(nb * NBS, NBS)],
                        start=False,
                        stop=(k == KC - 1),
                        reload_w=(nb == 0),
                    )

        def gemm_pair_fp8(psums, wA, rhsA, wB, rhsB):
            # fp8 DoubleRow: each matmul covers two 128-row contraction chunks
            for k in range(0, KC, 2):
                for nb in range(NB):
                    mm(
                        psums[nb][:],
                        wA[:, k : k + 2, :],
                        rhsA[:, k : k + 2, ds(nb * NBS, NBS)],
                        start=(k == 0),
                        stop=False,
                        reload_w=(nb == 0),
                        perf_mode=DR,
                    )
            for k in range(0, KC, 2):
                for nb in range(NB):
                    mm(
                        psums[nb][:],
                        wB[:, k : k + 2, :],
                        rhsB[:, k : k + 2, ds(nb * NBS, NBS)],
                        start=False,
                        stop=(k == KC - 2),
                        reload_w=(nb == 0),
                        perf_mode=DR,
                    )

        # ---- phase 1: r gate (hidden-major, fp8), rhT = sigmoid(...) * hT ----
        for jt in range(JT):
            if jt == 0:
                wr, ur = wr0, ur0
            else:
                wr = load_w("wr8", jt)
                ur = load_w("ur8", jt)
            # nb-outer so the first PSUM group only needs the first batch block
            ps = [
                psum.tile([128, NBS], F32, tag="ps", name=f"ps_r{jt}_{i}")
                for i in range(NB)
            ]
            for nb in range(NB):
                for k in range(0, KC, 2):
                    mm(
                        ps[nb][:],
                        wr[:, k : k + 2, :],
                        xT8[:, k : k + 2, ds(nb * NBS, NBS)],
                        start=(k == 0),
                        stop=False,
                        reload_w=False,
                        perf_mode=DR,
                    )
                for k in range(0, KC, 2):
                    mm(
                        ps[nb][:],
                        ur[:, k : k + 2, :],
                        hT8[:, k : k + 2, ds(nb * NBS, NBS)],
                        start=False,
                        stop=(k == KC - 2),
                        reload_w=False,
                        perf_mode=DR,
                    )
            for nb in range(NB):
                rt = actp.tile([128, NBS], BF16, tag="act")
                nc.scalar.activation(
                    rt[:],
                    ps[nb][:],
                    AFT.Sigmoid,
                    bias=bias_sb[:, 8 + jt : 9 + jt],
                    scale=INV_SCALE,
                )
                # rh in scaled fp8 for the DoubleRow A-matmul: (r*16)*h
                nc.vector.scalar_tensor_tensor(
                    rhT8[:, jt, ds(nb * NBS, NBS)],
                    rt[:],
                    SCALE_X,
                    hTb[:, jt, ds(nb * NBS, NBS)],
                    op0=ALU.mult,
                    op1=ALU.mult,
                )

        # ---- phase 2: z gate (fp8) + delta_h (bf16) + residual, per jt ----
        for jt in range(JT):
            wz = load_w("wz8", jt)
            uz = load_w("uz8", jt)
            vh = load_w("vhT", jt)
            at = load_w("at8", jt)
            psz = [
                psum.tile([128, NBS], F32, tag="ps", name=f"ps_z{jt}_{i}")
                for i in range(NB)
            ]
            gemm_pair_fp8(psz, wz, xT8, uz, hT8)
            # dh-pre: V_h@x in bf16 (V_h host-scaled by 4096) and A@(r*h) in
            # fp8 DoubleRow (scales 16*256) — separate PSUM groups (mixing
            # perf modes in one accumulation group faults the device), summed
            # on the DVE.
            psv = [
                psum.tile([128, NBS], F32, tag="ps", name=f"ps_v{jt}_{i}")
                for i in range(NB)
            ]
            for k in range(KC):
                for nb in range(NB):
                    mm(
                        psv[nb][:],
                        vh[:, k, :],
                        xT[:, k, ds(nb * NBS, NBS)],
                        start=(k == 0),
                        stop=(k == KC - 1),
                        reload_w=(nb == 0),
                    )
            vhs = []
            for nb in range(NB):
                t = tmpp.tile([128, NBS], F32, tag="vhs", name=f"vhs{jt}_{nb}")
                nc.vector.tensor_copy(t[:], psv[nb][:])
                vhs.append(t)
            psd = [
                psum.tile([128, NBS], F32, tag="ps", name=f"ps_d{jt}_{i}")
                for i in range(NB)
            ]
            for k in range(0, KC, 2):
                for nb in range(NB):
                    mm(
                        psd[nb][:],
                        at[:, k : k + 2, :],
                        rhT8[:, k : k + 2, ds(nb * NBS, NBS)],
                        start=(k == 0),
                        stop=(k == KC - 2),
                        reload_w=(nb == 0),
                        perf_mode=DR,
                    )
            ot = outp.tile([128, B], F32, tag="out")
            for nb in range(NB):
                zt = actp.tile([128, NBS], BF16, tag="act")
                nc.scalar.activation(
                    zt[:],
                    psz[nb][:],
                    AFT.Sigmoid,
                    bias=bias_sb[:, jt : jt + 1],
                    scale=INV_SCALE,
                )
                dsum = tmpp.tile([128, NBS], F32, tag="dsum")
                nc.vector.tensor_add(dsum[:], psd[nb][:], vhs[nb][:])
                dt_ = actp.tile([128, NBS], BF16, tag="act")
                nc.scalar.activation(
                    dt_[:],
                    dsum[:],
                    AFT.Tanh,
                    bias=bias_sb[:, 16 + jt : 17 + jt],
                    scale=INV_SCALE,
                )
                zdh = tmpp.tile([128, NBS], F32, tag="zdh")
                nc.vector.tensor_mul(zdh[:], zt[:], dt_[:])
                # out = (z*dh) * eps + h
                nc.vector.scalar_tensor_tensor(
                    ot[:, ds(nb * NBS, NBS)],
                    zdh[:],
                    float(eps),
                    hTb[:, jt, ds(nb * NBS, NBS)],
                    op0=ALU.mult,
                    op1=ALU.add,
                )
                nc.sync.dma_start(
                    out=out_d[ts(jt, 128), ds(nb * NBS, NBS)],
                    in_=ot[:, ds(nb * NBS, NBS)],
                )

    _dedupe_ldweights(nc)
    nc.compile()
    return nc


def _dedupe_ldweights(nc):
    """Drop back-to-back InstLdweights with identical weight APs.

    Tile legalization splits every bf16 matmul into LDWEIGHTS+MATMUL even when
    consecutive matmuls share the stationary operand. The PE executes its
    stream in order, so a repeated load of the same weights is pure overhead
    (~128 cycles per 512-cycle matmul). Only drops loads that carry no
    semaphore waits/updates; the explicit pe-order dep chain built in _build
    guarantees groups sharing weights are contiguous in the stream.
    """
    removed = 0
    for blk in nc.m.functions[0].blocks:
        new = []
        last_key = None
        for i in blk.instructions:
            if i.engine == mybir.EngineType.PE:
                if isinstance(i, mybir.InstLdweights):
                    si = i.sync_info
                    clean = si is None or (not si.on_wait and not si.on_update)
                    key = str(i.ins[0])
                    if clean and key == last_key:
                        removed += 1
                        continue
                    last_key = key
                elif not isinstance(i, mybir.InstMatmult):
                    last_key = None
            new.append(i)
        blk.instructions[:] = new
    return removed


def _get_nc(eps: float):
    key = float(eps)
    if key not in _nc_cache:
        _nc_cache[key] = _build(key)
    return _nc_cache[key]


def _block_weight(wT, dtype, scale=1.0):
    # [1024, 1024] (contraction-major) -> [jt, p, c, j] st. blk[jt,p,c,j] = wT[c*128+p, jt*128+j]
    blk = wT.reshape(KC, 128, JT, 128).transpose(2, 1, 0, 3)
    if scale != 1.0:
        blk = blk * scale
    return np.ascontiguousarray(blk).astype(dtype)


def _block_data(m):
    # per-core [B, 1024] -> [p, c, b] st. blk[p,c,b] = m[b, c*128+p]
    return np.ascontiguousarray(m.T.reshape(KC, 128, B).transpose(1, 0, 2))


def _block_data_nb(m):
    # per-core [B, 1024] -> [nb, p, c, nbs] batch-block-major
    blk = m.T.reshape(KC, 128, NB, NBS).transpose(2, 1, 0, 3)
    return np.ascontiguousarray(blk)


def _prep_in_maps(x, h_prev, W_z, b_z, U_z, W_r, b_r, U_r, V_h, b_h, W_h):
    BF = ml_dtypes.bfloat16
    F8 = ml_dtypes.float8_e4m3
    x16 = np.asarray(x, np.float32).astype(BF)
    h16 = np.asarray(h_prev, np.float32).astype(BF)
    x8 = (np.asarray(x, np.float32) * SCALE_X).astype(F8)
    h8 = (np.asarray(h_prev, np.float32) * SCALE_X).astype(F8)

    A = W_h - W_h.T - GAMMA * np.eye(H, dtype=np.float32)
    shared = {
        "wz8": _block_weight(W_z.T, F8, SCALE_W),
        "uz8": _block_weight(U_z.T, F8, SCALE_W),
        "wr8": _block_weight(W_r.T, F8, SCALE_W),
        "ur8": _block_weight(U_r.T, F8, SCALE_W),
        "at8": _block_weight(A.T, F8, SCALE_W),
        "vhT": _block_weight(V_h.T, BF, SCALE_X * SCALE_W),
        "biases": np.ascontiguousarray(
            np.concatenate(
                [
                    b_z.reshape(JT, 128).T,
                    b_r.reshape(JT, 128).T,
                    b_h.reshape(JT, 128).T,
                ],
                axis=1,
            ).astype(np.float32)
        ),
    }
    in_maps = []
    for c in range(N_CORES):
        sl = slice(c * B, (c + 1) * B)
        in_maps.append(
            {
                "xT": _block_data(x16[sl]),
                "hT": _block_data(h16[sl]),
                "xT8": _block_data_nb(x8[sl]),
                "hT8": _block_data_nb(h8[sl]),
                **shared,
            }
        )
    return in_maps


def run(inputs, trace=False):
    """Returns (full_output [16384,1024] f32, BassKernelResults)."""
    np_in = {k: np.asarray(v, np.float32) for k, v in inputs.items()}
    eps = float(np_in.pop("epsilon"))
    in_maps = _prep_in_maps(**np_in)
    nc = _get_nc(eps)
    res = run_bass_kernel_spmd(
        nc, in_maps, core_ids=list(range(N_CORES)), trace=trace
    )
    out = np.empty((BATCH, H), np.float32)
    for c in range(N_CORES):
        out[c * B : (c + 1) * B, :] = res.results[c]["out"].T
    return out, res


def kernel(**inputs) -> np.ndarray:
    out, _ = run(inputs, trace=False)
    return out


# revision 44
# speedup vs baseline: 2.1816x; 1.1776x over previous
"""AGRU cell (antisymmetric GRU) forward on 8 TRN2 NeuronCores.

Data-parallel: batch 16384 is sharded 2048 rows/core; the six 1024x1024
weight matrices are replicated. No cross-core communication.

Everything on-device is computed in "hidden-major" (transposed) layout:
    zT = sigmoid(Wz @ xT + Uz @ hT + bz)        [H, B]
    rT = sigmoid(Wr @ xT + Ur @ hT + br)
    rhT = rT * hT
    dhT = tanh(Vh @ xT + A @ rhT + bh)
    outT = hT + eps * zT * dhT
so every matmul has the (pre-transposed, host-prepared) weight tile as the
stationary operand and xT/hT/rhT as the moving operand, and nothing ever
needs an on-device transpose.  The host transposes each core's [1024, 2048]
result back when assembling the full output.

Matmuls run in bf16 (1 cycle/row on TRN2 vs 4 for fp32) with fp32 PSUM
accumulation; the final residual add is done in fp32.
"""

import sys

sys.path.insert(0, "/opt/trn_rl_repo")

import numpy as np
import ml_dtypes

from contextlib import ExitStack

import concourse.bass as bass
import concourse.mybir as mybir
from concourse import bacc, tile
from concourse.bass import ds, ts
from concourse.bass_utils import run_bass_kernel_spmd
from concourse.tile_rust import add_dep_helper

BF16 = mybir.dt.bfloat16
FP8 = mybir.dt.float8e4
F32 = mybir.dt.float32
AFT = mybir.ActivationFunctionType
ALU = mybir.AluOpType
DR = mybir.MatmulPerfMode.DoubleRow

# fp8 pre-scaling for the sigmoid-gate GEMMs (z, r): data*16, weights*256,
# compensated by activation scale 1/(16*256).
SCALE_X = 16.0
SCALE_W = 256.0
INV_SCALE = 1.0 / (SCALE_X * SCALE_W)

N_CORES = 8
BATCH = 16384
B = BATCH // N_CORES  # per-core batch shard (2048)
H = 1024  # hidden == input size
KC = H // 128  # contraction chunks (8)
JT = H // 128  # output row tiles (8)
NB = 4  # moving-dim (batch) blocks per psum bank
NBS = B // NB  # 512 columns per matmul
GAMMA = 0.01

_nc_cache = {}


def _build(eps: float):
    """Build + compile the single-core Tile program (same graph on all cores)."""
    nc = bacc.Bacc("TRN2", target_bir_lowering=False, debug=False)

    hT_d = nc.dram_tensor("hT", [128, KC, B], BF16, kind="ExternalInput")
    # fp8 activations arrive batch-block-major so phase 1 can start on the
    # first 512-column block instead of waiting for the full stream.
    xT8_d = nc.dram_tensor("xT8", [NB, 128, KC, NBS], FP8, kind="ExternalInput")
    hT8_d = nc.dram_tensor("hT8", [NB, 128, KC, NBS], FP8, kind="ExternalInput")
    w8_d = {
        name: nc.dram_tensor(name, [JT, 128, KC, 128], FP8, kind="ExternalInput")
        for name in ["wz8", "uz8", "wr8", "ur8", "at8", "vh8"]
    }
    bias_d = nc.dram_tensor("biases", [128, 24], F32, kind="ExternalInput")
    out_d = nc.dram_tensor("out", [H, B], F32, kind="ExternalOutput")

    with tile.TileContext(nc) as tc, ExitStack() as ctx:
        singles = ctx.enter_context(tc.tile_pool(name="singles", bufs=1))
        wpool = ctx.enter_context(tc.tile_pool(name="wpool", bufs=8))
        psum = ctx.enter_context(tc.tile_pool(name="psum", bufs=8, space="PSUM"))
        actp = ctx.enter_context(tc.tile_pool(name="actp", bufs=10))
        tmpp = ctx.enter_context(tc.tile_pool(name="tmpp", bufs=5))
        outp = ctx.enter_context(tc.tile_pool(name="outp", bufs=2))

        hTb = singles.tile([128, KC, B], BF16)
        xT8 = singles.tile([128, KC, B], FP8)
        hT8 = singles.tile([128, KC, B], FP8)
        rhT8 = singles.tile([128, KC, B], FP8)
        bias_sb = singles.tile([128, 24], F32)

        def load_w(name, jt):
            # gpsimd (SWDGE): keeps weight blocks off the Sync HWDGE queue,
            # which is busy issuing the bulk x/h stream.
            w = wpool.tile([128, KC, 128], FP8, tag="w")
            nc.gpsimd.dma_start(out=w[:], in_=w8_d[name][jt])
            return w

        # Weights for the first row-block go out first so the PE isn't stuck
        # behind the x/h stream on the DMA queues.
        wr0 = load_w("wr8", 0)
        ur0 = load_w("ur8", 0)
        for nb in range(NB):
            nc.sync.dma_start(out=xT8[:, :, ds(nb * NBS, NBS)], in_=xT8_d[nb])
            nc.sync.dma_start(out=hT8[:, :, ds(nb * NBS, NBS)], in_=hT8_d[nb])
        nc.sync.dma_start(out=bias_sb[:], in_=bias_d[:])
        # bf16 h: phase-1's rh-multiplies and the residual consume it.
        for c in range(KC):
            nc.sync.dma_start(out=hTb[:, c, :], in_=hT_d[:, c, :])

        # All PE matmuls are chained in program order (ordering-only deps) so
        # that groups of matmuls sharing a stationary operand stay contiguous:
        # followers in each group skip their LDWEIGHTS (ldweights=False) and
        # reuse the weights already in the array.
        prev_mm = [None]

        def mm(psum_ap, w_ap, rhs_ap, start, stop, reload_w, perf_mode=None):
            bi = nc.tensor.matmul(
                psum_ap, w_ap, rhs_ap, start=start, stop=stop, perf_mode=perf_mode
            )
            if not reload_w:
                bi.ins.ldweights = False
            if prev_mm[0] is not None:
                add_dep_helper(bi.ins, prev_mm[0], False, "pe-order")
            prev_mm[0] = bi.ins
            return bi

        def gemm_pair(psums, wA, rhsA, wB, rhsB):
            # psums[nb] += wA[:,k,:].T @ rhsA[:,k,nb] summed over k, then wB/rhsB
            for k in range(KC):
                for nb in range(NB):
                    mm(
                        psums[nb][:],
                        wA[:, k, :],
                        rhsA[:, k, ds(nb * NBS, NBS)],
                        start=(k == 0),
                        stop=False,
                        reload_w=(nb == 0),
                    )
            for k in range(KC):
                for nb in range(NB):
                    mm(
                        psums[nb][:],
                        wB[:, k, :],
                        rhsB[:, k, ds(nb * NBS, NBS)],
                        start=False,
                        stop=(k == KC - 1),
                        reload_w=(nb == 0),
                    )

        def gemm_pair_fp8(psums, wA, rhsA, wB, rhsB):
            # fp8 DoubleRow: each matmul covers two 128-row contraction chunks
            for k in range(0, KC, 2):
                for nb in range(NB):
                    mm(
                        psums[nb][:],
                        wA[:, k : k + 2, :],
                        rhsA[:, k : k + 2, ds(nb * NBS, NBS)],
                        start=(k == 0),
                        stop=False,
                        reload_w=(nb == 0),
                        perf_mode=DR,
                    )
            for k in range(0, KC, 2):
                for nb in range(NB):
                    mm(
                        psums[nb][:],
                        wB[:, k : k + 2, :],
                        rhsB[:, k : k + 2, ds(nb * NBS, NBS)],
                        start=False,
                        stop=(k == KC - 2),
                        reload_w=(nb == 0),
                        perf_mode=DR,
                    )

        # ---- phase 1: r gate (hidden-major, fp8), rhT = sigmoid(...) * hT ----
        for jt in range(JT):
            if jt == 0:
                wr, ur = wr0, ur0
            else:
                wr = load_w("wr8", jt)
                ur = load_w("ur8", jt)
            # nb-outer so the first PSUM group only needs the first batch block
            ps = [
                psum.tile([128, NBS], F32, tag="ps", name=f"ps_r{jt}_{i}")
                for i in range(NB)
            ]
            for nb in range(NB):
                for k in range(0, KC, 2):
                    mm(
                        ps[nb][:],
                        wr[:, k : k + 2, :],
                        xT8[:, k : k + 2, ds(nb * NBS, NBS)],
                        start=(k == 0),
                        stop=False,
                        reload_w=False,
                        perf_mode=DR,
                    )
                for k in range(0, KC, 2):
                    mm(
                        ps[nb][:],
                        ur[:, k : k + 2, :],
                        hT8[:, k : k + 2, ds(nb * NBS, NBS)],
                        start=False,
                        stop=(k == KC - 2),
                        reload_w=False,
                        perf_mode=DR,
                    )
            for nb in range(NB):
                rt = actp.tile([128, NBS], BF16, tag="act")
                nc.scalar.activation(
                    rt[:],
                    ps[nb][:],
                    AFT.Sigmoid,
                    bias=bias_sb[:, 8 + jt : 9 + jt],
                    scale=INV_SCALE,
                )
                # rh in scaled fp8 for the DoubleRow A-matmul: (r*16)*h
                nc.vector.scalar_tensor_tensor(
                    rhT8[:, jt, ds(nb * NBS, NBS)],
                    rt[:],
                    SCALE_X,
                    hTb[:, jt, ds(nb * NBS, NBS)],
                    op0=ALU.mult,
                    op1=ALU.mult,
                )

        # ---- phase 2: z gate (fp8) + delta_h (bf16) + residual, per jt ----
        for jt in range(JT):
            wz = load_w("wz8", jt)
            uz = load_w("uz8", jt)
            vh = load_w("vh8", jt)
            at = load_w("at8", jt)
            psz = [
                psum.tile([128, NBS], F32, tag="ps", name=f"ps_z{jt}_{i}")
                for i in range(NB)
            ]
            gemm_pair_fp8(psz, wz, xT8, uz, hT8)
            # dh-pre: V_h@x + A@(r*h), both fp8 DoubleRow at combined scale
            # 16*256, one PSUM accumulation group.
            psd = [
                psum.tile([128, NBS], F32, tag="ps", name=f"ps_d{jt}_{i}")
                for i in range(NB)
            ]
            gemm_pair_fp8(psd, vh, xT8, at, rhT8)
            ot = outp.tile([128, B], F32, tag="out")
            for nb in range(NB):
                zt = actp.tile([128, NBS], BF16, tag="act")
                nc.scalar.activation(
                    zt[:],
                    psz[nb][:],
                    AFT.Sigmoid,
                    bias=bias_sb[:, jt : jt + 1],
                    scale=INV_SCALE,
                )
                dt_ = actp.tile([128, NBS], BF16, tag="act")
                nc.scalar.activation(
                    dt_[:],
                    psd[nb][:],
                    AFT.Tanh,
                    bias=bias_sb[:, 16 + jt : 17 + jt],
                    scale=INV_SCALE,
                )
                zdh = tmpp.tile([128, NBS], F32, tag="zdh")
                nc.vector.tensor_mul(zdh[:], zt[:], dt_[:])
                # out = (z*dh) * eps + h
                nc.vector.scalar_tensor_tensor(
                    ot[:, ds(nb * NBS, NBS)],
                    zdh[:],
                    float(eps),
                    hTb[:, jt, ds(nb * NBS, NBS)],
                    op0=ALU.mult,
                    op1=ALU.add,
                )
                nc.sync.dma_start(
                    out=out_d[ts(jt, 128), ds(nb * NBS, NBS)],
                    in_=ot[:, ds(nb * NBS, NBS)],
                )

    _dedupe_ldweights(nc)
    nc.compile()
    return nc


def _dedupe_ldweights(nc):
    """Drop back-to-back InstLdweights with identical weight APs.

    Tile legalization splits every bf16 matmul into LDWEIGHTS+MATMUL even when
    consecutive matmuls share the stationary operand. The PE executes its
    stream in order, so a repeated load of the same weights is pure overhead
    (~128 cycles per 512-cycle matmul). Only drops loads that carry no
    semaphore waits/updates; the explicit pe-order dep chain built in _build
    guarantees groups sharing weights are contiguous in the stream.
    """
    removed = 0
    for blk in nc.m.functions[0].blocks:
        new = []
        last_key = None
        for i in blk.instructions:
            if i.engine == mybir.EngineType.PE:
                if isinstance(i, mybir.InstLdweights):
                    si = i.sync_info
                    clean = si is None or (not si.on_wait and not si.on_update)
                    key = str(i.ins[0])
                    if clean and key == last_key:
                        removed += 1
                        continue
                    last_key = key
                elif not isinstance(i, mybir.InstMatmult):
                    last_key = None
            new.append(i)
        blk.instructions[:] = new
    return removed


def _get_nc(eps: float):
    key = float(eps)
    if key not in _nc_cache:
        _nc_cache[key] = _build(key)
    return _nc_cache[key]


def _block_weight(wT, dtype, scale=1.0):
    # [1024, 1024] (contraction-major) -> [jt, p, c, j] st. blk[jt,p,c,j] = wT[c*128+p, jt*128+j]
    blk = wT.reshape(KC, 128, JT, 128).transpose(2, 1, 0, 3)
    if scale != 1.0:
        blk = blk * scale
    return np.ascontiguousarray(blk).astype(dtype)


def _block_data(m):
    # per-core [B, 1024] -> [p, c, b] st. blk[p,c,b] = m[b, c*128+p]
    return np.ascontiguousarray(m.T.reshape(KC, 128, B).transpose(1, 0, 2))


def _block_data_nb(m):
    # per-core [B, 1024] -> [nb, p, c, nbs] batch-block-major
    blk = m.T.reshape(KC, 128, NB, NBS).transpose(2, 1, 0, 3)
    return np.ascontiguousarray(blk)


def _prep_in_maps(x, h_prev, W_z, b_z, U_z, W_r, b_r, U_r, V_h, b_h, W_h):
    BF = ml_dtypes.bfloat16
    F8 = ml_dtypes.float8_e4m3
    h16 = np.asarray(h_prev, np.float32).astype(BF)
    x8 = (np.asarray(x, np.float32) * SCALE_X).astype(F8)
    h8 = (np.asarray(h_prev, np.float32) * SCALE_X).astype(F8)

    A = W_h - W_h.T - GAMMA * np.eye(H, dtype=np.float32)
    shared = {
        "wz8": _block_weight(W_z.T, F8, SCALE_W),
        "uz8": _block_weight(U_z.T, F8, SCALE_W),
        "wr8": _block_weight(W_r.T, F8, SCALE_W),
        "ur8": _block_weight(U_r.T, F8, SCALE_W),
        "at8": _block_weight(A.T, F8, SCALE_W),
        "vh8": _block_weight(V_h.T, F8, SCALE_W),
        "biases": np.ascontiguousarray(
            np.concatenate(
                [
                    b_z.reshape(JT, 128).T,
                    b_r.reshape(JT, 128).T,
                    b_h.reshape(JT, 128).T,
                ],
                axis=1,
            ).astype(np.float32)
        ),
    }
    in_maps = []
    for c in range(N_CORES):
        sl = slice(c * B, (c + 1) * B)
        in_maps.append(
            {
                "hT": _block_data(h16[sl]),
                "xT8": _block_data_nb(x8[sl]),
                "hT8": _block_data_nb(h8[sl]),
                **shared,
            }
        )
    return in_maps


def run(inputs, trace=False):
    """Returns (full_output [16384,1024] f32, BassKernelResults)."""
    np_in = {k: np.asarray(v, np.float32) for k, v in inputs.items()}
    eps = float(np_in.pop("epsilon"))
    in_maps = _prep_in_maps(**np_in)
    nc = _get_nc(eps)
    res = run_bass_kernel_spmd(
        nc, in_maps, core_ids=list(range(N_CORES)), trace=trace
    )
    out = np.empty((BATCH, H), np.float32)
    for c in range(N_CORES):
        out[c * B : (c + 1) * B, :] = res.results[c]["out"].T
    return out, res


def kernel(**inputs) -> np.ndarray:
    out, _ = run(inputs, trace=False)
    return out
